# revision 10
# baseline (speedup 1.0000x reference)
"""Trainium2 Bass kernel for one pre-LN transformer block (B=8, T=1024, C=256,
H=16 heads of size 16, FFN 256->1024->256), data-parallel over batch across 8
NeuronCores (one batch element per core).

Per-core dataflow (matmul operands bf16, accumulation fp32):
  x arrives bf16 per-tile on three DMA queues (sync/scalar/gpsimd) so LN1
    starts ~9us in; LN affines are folded into the QKV/FFN1 weights host-side
  LN1 (straight [T,C]; batched Quake-rsqrt on DVE, apply+evacs on ScalarE
    with per-partition scale/bias) -> PE-transpose -> h1^T [C,T]
  Q^T/K^T in padded head layout [h*32+d, T] (pad rows zero, padded weights)
  V straight [T, h*32+{d,16=ones-col,zeros}] - the ones column makes the PV
    matmul also produce the softmax denominator (scores are tiny: no max pass)
  S^T[tk,tq] = k^T.T @ q^T per head via 32-row-strip matmuls (4 heads share
    the 128-row PE array); one PSUM alloc per head-PAIR so the ring pipelines
    S^T(i+1) against exp(i); exp fused into the PSUM->SBUF evac: heads 0-1
    table-exp on ScalarE, heads 2-3 Schraudolph bf16-bits exp on VectorE;
    causal diag blocks masked by one batched diagonal-strided triangular
    multiply per (pack, head) against a single [128,128] mask read 4x via a
    stride-0 middle AP dim
  PV: out^T[d,tq] accumulated over tk tiles with 32-col-strip matmuls
  normalize via per-head Z row broadcast (stream_shuffle from PSUM) +
    reciprocal_approx_fast + multiply (the last unit runs this in 128-col
    pieces so the projection can start early)
  proj: x1 = x(+bp) + out^T.T @ Wp   (out^T tiles are the stationary operand)
  LN2 -> h2^T -> FFN1 (relu+bias on ScalarE evac) -> FFN2 -> + x1

Scheduling: program order interleaves chunk production with attention so the
PE never waits on a full-phase barrier, and independent GEMMs are dropped
into attention stall points as "fillers" (QK chunk-1 packs into attention
chunk 0, FFN1 chunk-0 tiles into attention chunk 1) to keep the tensor
engine dense while exp evacuations drain.
"""

import os
import sys

for _p in ("/opt/trn_rl_repo", "/root/.axon_site/_ro/trn_rl_repo"):
    if os.path.isdir(_p) and _p not in sys.path:
        sys.path.append(_p)

import numpy as np
import ml_dtypes

# problem shapes (hardcoded per contest rules)
B, T, C, H, D, F = 8, 1024, 256, 16, 16, 1024
P = 128          # partitions
NT = T // P      # 8 T-tiles
HP = 32          # padded per-head stride (Q/K/V/out layouts)
CP = H * HP      # 512 padded channel dim
NPACK = 4        # head packs (4 heads per 128-partition tile)
NKC = C // P     # 2 k-tiles over C
EPS = 1e-5
SCALE = D ** -0.5
MAGIC = 0x5F3759DF
# Schraudolph-style exp to bf16 bits: bf16_bits(exp(SCALE*s)) ~= EXP_A*s + EXP_B
EXP_A = (2 ** 7) * SCALE * 1.4426950408889634
EXP_B = 2 ** 7 * 127 - 5.6

_BF16 = ml_dtypes.bfloat16

_cache = {}


def _build_program(qk_bias=False, bp_zero=False, b2_zero=False, b1_zero=False):
    import concourse.bass as bass
    import concourse.bacc as bacc
    import concourse.tile as tile
    import concourse.mybir as mybir

    dt = mybir.dt
    f32, bf16, i32, i16 = dt.float32, dt.bfloat16, dt.int32, dt.int16
    AF = mybir.ActivationFunctionType
    ALU = mybir.AluOpType

    nc = bacc.Bacc("TRN2", target_bir_lowering=False, debug=False)

    # ---- DRAM I/O ----
    x_d = nc.dram_tensor("x", [T, C], bf16, kind="ExternalInput")
    wq_d = nc.dram_tensor("wq", [C, CP], bf16, kind="ExternalInput")
    wk_d = nc.dram_tensor("wk", [C, CP], bf16, kind="ExternalInput")
    wv_d = nc.dram_tensor("wv", [C, CP], bf16, kind="ExternalInput")
    wp_d = nc.dram_tensor("wp", [CP, C], bf16, kind="ExternalInput")
    w1_d = nc.dram_tensor("w1", [C, F], bf16, kind="ExternalInput")
    w2_d = nc.dram_tensor("w2", [F, C], bf16, kind="ExternalInput")
    bq_d = nc.dram_tensor("bq", [CP], f32, kind="ExternalInput")
    bk_d = nc.dram_tensor("bk", [CP], f32, kind="ExternalInput")
    bp_d = nc.dram_tensor("bprow", [C], f32, kind="ExternalInput")
    b1_d = nc.dram_tensor("b1p", [F], f32, kind="ExternalInput")
    b2_d = nc.dram_tensor("b2row", [C], f32, kind="ExternalInput")
    out_d = nc.dram_tensor("out", [T, C], f32, kind="ExternalOutput")

    # S^T diag tile mask: partition = tk local, free = tq local; keep tq >= tk
    tri_np = np.triu(np.ones((P, P), dtype=np.float32)).astype(_BF16)
    tri_d = nc.inline_tensor(tri_np, name="trimask")

    with tile.TileContext(nc) as tc:
        consts = tc.alloc_tile_pool(name="consts", bufs=1)
        data = tc.alloc_tile_pool(name="data", bufs=1)
        attn = tc.alloc_tile_pool(name="attn", bufs=1)
        work = tc.alloc_tile_pool(name="work", bufs=4)
        psum = tc.alloc_tile_pool(name="psum", bufs=1, space="PSUM")

        # ---- persistent SBUF tensors ----
        tri_s = consts.tile([P, P], bf16)
        wq_s = consts.tile([P, NKC, CP], bf16)
        wk_s = consts.tile([P, NKC, CP], bf16)
        wv_s = consts.tile([P, NKC, CP], bf16)
        wp_s = consts.tile([P, NPACK, C], bf16)
        w1_s = consts.tile([P, NKC, F], bf16)
        w2_s = consts.tile([P, NT, C], bf16)
        bq_s = consts.tile([P, NPACK], f32)
        bk_s = consts.tile([P, NPACK], f32)
        b1_s = consts.tile([P, NT], f32)

        xs = data.tile([P, NT, C], bf16)
        xbp = xs if bp_zero else data.tile([P, NT, C], bf16)
        h1T = data.tile([P, NKC, T], bf16)
        QT = data.tile([P, NPACK, T], bf16)
        KT = data.tile([P, NPACK, T], bf16)
        Vv = data.tile([P, NT, CP], bf16)
        OUTT = data.tile([P, NPACK, T], bf16)
        x1 = data.tile([P, NT, C], f32)
        h2T = data.tile([P, NKC, T], bf16)
        HT = data.tile([P, NT, F], bf16)

        # ---- input DMAs: x tiles first on the two fast HWDGE queues
        # (sync/scalar) so LN1 starts ASAP; weights go on the gpsimd SWDGE
        # queue, which also leaves the gpsimd engine free afterwards ----
        x_r = x_d[:, :].rearrange("(j p) c -> p j c", p=P)
        x_engines = [nc.sync, nc.scalar, nc.sync, nc.scalar,
                     nc.sync, nc.scalar, nc.sync, nc.scalar]
        for j in range(NT):
            x_engines[j].dma_start(out=xs[:, j], in_=x_r[:, j])
        nc.gpsimd.dma_start(out=wq_s, in_=wq_d[:, :].rearrange("(k p) c -> p k c", p=P))
        nc.gpsimd.dma_start(out=wk_s, in_=wk_d[:, :].rearrange("(k p) c -> p k c", p=P))
        nc.gpsimd.dma_start(out=wv_s, in_=wv_d[:, :].rearrange("(k p) c -> p k c", p=P))
        nc.gpsimd.dma_start(out=tri_s, in_=tri_d[:, :])
        nc.gpsimd.dma_start(out=wp_s, in_=wp_d[:, :].rearrange("(k p) c -> p k c", p=P))
        nc.scalar.dma_start(out=w1_s, in_=w1_d[:, :].rearrange("(k p) c -> p k c", p=P))
        nc.sync.dma_start(out=w2_s, in_=w2_d[:, :].rearrange("(k p) c -> p k c", p=P))
        if not bp_zero:
            nc.scalar.dma_start(
                out=xbp, in_=x_d[:, :].rearrange("(j p) c -> p j c", p=P))
            # add bp (broadcast along partitions) into the residual copy
            bp_b = bass.AP(tensor=bp_d, offset=0, ap=[[0, P], [1, C]])
            bpt = consts.tile([P, C], f32)
            nc.sync.dma_start(out=bpt, in_=bp_b)
            for j in range(NT):
                nc.vector.tensor_add(out=xbp[:, j], in0=xbp[:, j], in1=bpt)
        if qk_bias:
            nc.sync.dma_start(out=bq_s, in_=bq_d[:].rearrange("(m p) -> p m", p=P))
            nc.sync.dma_start(out=bk_s, in_=bk_d[:].rearrange("(m p) -> p m", p=P))
        if not b1_zero:
            nc.sync.dma_start(out=b1_s, in_=b1_d[:].rearrange("(m p) -> p m", p=P))
        if not b2_zero:
            b2t = consts.tile([P, C], f32)
            b2_b = bass.AP(tensor=b2_d, offset=0, ap=[[0, P], [1, C]])
            nc.sync.dma_start(out=b2t, in_=b2_b)

        def ln_phase(src, dst_hT, tag, tiles, sc_stats=False):
            """LayerNorm the given tiles of src [128, 8, 256] f32 and write
            the transposed bf16 result into dst_hT [128, 2, 1024].
            sc_stats: compute Σx/Σx² on ScalarE (accum_out) instead of DVE
            bn_stats — used where the DVE is the exposed critical path."""
            nj = len(tiles)
            mvall = work.tile([P, nj, 2], f32, tag="mvall", name=f"mv_{tag}")
            vpe = work.tile([P, nj], f32, tag="vpe", name=f"vpe_{tag}")
            if sc_stats:
                sx = work.tile([P, nj], f32, tag="sx", name=f"sx_{tag}")
                sxx = work.tile([P, nj], f32, tag="sxx", name=f"sxx_{tag}")
                for jx, j in enumerate(tiles):
                    scr = work.tile([P, C], f32, tag="scr")
                    nc.scalar.activation(
                        out=scr, in_=src[:, j], func=AF.Identity,
                        accum_out=sx[:, jx : jx + 1])
                    scr2 = work.tile([P, C], f32, tag="scr")
                    nc.scalar.activation(
                        out=scr2, in_=src[:, j], func=AF.Square,
                        accum_out=sxx[:, jx : jx + 1])
                nc.vector.tensor_scalar(
                    out=mvall[:, :, 0], in0=sx, scalar1=1.0 / C, scalar2=None,
                    op0=ALU.mult)
                mm = work.tile([P, nj], f32, tag="mm2", name=f"mm2_{tag}")
                nc.vector.tensor_tensor(
                    out=mm, in0=mvall[:, :, 0], in1=mvall[:, :, 0], op=ALU.mult)
                nc.vector.tensor_scalar(
                    out=vpe, in0=sxx, scalar1=1.0 / C, scalar2=EPS,
                    op0=ALU.mult, op1=ALU.add)
                nc.vector.tensor_tensor(
                    out=vpe, in0=vpe, in1=mm, op=ALU.subtract)
            else:
                for jx, j in enumerate(tiles):
                    stats = work.tile([P, 6], f32, tag="stats")
                    nc.vector.bn_stats(out=stats, in_=src[:, j])
                    nc.vector.bn_aggr(out=mvall[:, jx], in_=stats)
                nc.vector.tensor_scalar_add(
                    out=vpe, in0=mvall[:, :, 1], scalar1=EPS)
            # rstd for all tiles: Quake rsqrt + 2 Newton steps (pure DVE)
            sh = work.tile([P, nj], i32, tag="rsq_sh")
            nc.vector.tensor_scalar(
                out=sh, in0=vpe.bitcast(i32), scalar1=1, scalar2=None,
                op0=ALU.logical_shift_right,
            )
            y0 = work.tile([P, nj], i32, tag="rsq_y0")
            nc.vector.tensor_scalar(
                out=y0, in0=sh, scalar1=-1, scalar2=MAGIC,
                op0=ALU.mult, op1=ALU.add,
            )
            y = y0.bitcast(f32)
            rsq = work.tile([P, nj], f32, tag="rsq", name=f"rsq_{tag}")
            tmp = work.tile([P, nj], f32, tag="rsq_tmp")
            for it in range(2):
                nc.vector.tensor_tensor(out=tmp, in0=y, in1=y, op=ALU.mult)
                nc.vector.tensor_tensor(out=tmp, in0=tmp, in1=vpe, op=ALU.mult)
                nc.vector.tensor_scalar(
                    out=tmp, in0=tmp, scalar1=-0.5, scalar2=1.5,
                    op0=ALU.mult, op1=ALU.add,
                )
                nc.vector.tensor_tensor(out=rsq, in0=tmp, in1=y, op=ALU.mult)
                y = rsq
            # bias for the ScalarE apply: -mu*rstd
            nmurs = work.tile([P, nj], f32, tag="nmurs", name=f"nmurs_{tag}")
            nc.vector.tensor_tensor(
                out=nmurs, in0=mvall[:, :, 0], in1=rsq, op=ALU.mult)
            nc.vector.tensor_scalar(
                out=nmurs, in0=nmurs, scalar1=-1.0, scalar2=None, op0=ALU.mult)
            for jx, j in enumerate(tiles):
                # apply on GpSimd (SBUF->SBUF, frees ScalarE), transpose on
                # the DMA xbar engine (frees the PE and the PSUM evac)
                hs = work.tile([P, C], bf16, tag="hstraight")
                nc.gpsimd.tensor_scalar(
                    out=hs, in0=src[:, j],
                    scalar1=rsq[:, jx : jx + 1], scalar2=nmurs[:, jx : jx + 1],
                    op0=ALU.mult, op1=ALU.add,
                )
                for k in range(NKC):
                    eng = nc.sync if (j + k) % 2 == 0 else nc.scalar
                    eng.dma_start_transpose(
                        out=dst_hT[:, k, j * P : (j + 1) * P],
                        in_=hs[:, k * P : (k + 1) * P],
                    )

        def qk_pack(w_s, b_s, dstT, m, c):
            ps = psum.tile([P, 512], f32, tag="mix", bufs=2)
            for k in range(NKC):
                nc.tensor.matmul(
                    ps,
                    lhsT=w_s[:, k, m * P : (m + 1) * P],
                    rhs=h1T[:, k, c * 512 : (c + 1) * 512],
                    start=(k == 0), stop=(k == NKC - 1),
                )
            if qk_bias:
                nc.vector.tensor_scalar_add(
                    out=dstT[:, m, c * 512 : (c + 1) * 512], in0=ps,
                    scalar1=b_s[:, m : m + 1],
                )
            else:
                nc.scalar.activation(
                    out=dstT[:, m, c * 512 : (c + 1) * 512], in_=ps,
                    func=AF.Copy,
                )

        def ffn1_tile(f, c):
            ps = psum.tile([P, 512], f32, tag="mix", bufs=2)
            for k in range(NKC):
                nc.tensor.matmul(
                    ps,
                    lhsT=w1_s[:, k, f * P : (f + 1) * P],
                    rhs=h2T[:, k, c * 512 : (c + 1) * 512],
                    start=(k == 0), stop=(k == NKC - 1),
                )
            if b1_zero:
                nc.scalar.activation(
                    out=HT[:, f, c * 512 : (c + 1) * 512], in_=ps, func=AF.Relu)
            else:
                nc.scalar.activation(
                    out=HT[:, f, c * 512 : (c + 1) * 512], in_=ps,
                    func=AF.Relu, bias=b1_s[:, f : f + 1],
                )

        def qk_chunk(c):
            """Q^T / K^T for tq-chunk c (padded layout, bias folded in evac)."""
            for (name, w_s, b_s, dstT) in (
                ("q", wq_s, bq_s, QT), ("k", wk_s, bk_s, KT)):
                for m in range(NPACK):
                    ps = psum.tile([P, 512], f32, tag="mix", bufs=2)
                    for k in range(NKC):
                        nc.tensor.matmul(
                            ps,
                            lhsT=w_s[:, k, m * P : (m + 1) * P],
                            rhs=h1T[:, k, c * 512 : (c + 1) * 512],
                            start=(k == 0), stop=(k == NKC - 1),
                        )
                    if qk_bias:
                        nc.vector.tensor_scalar_add(
                            out=dstT[:, m, c * 512 : (c + 1) * 512], in0=ps,
                            scalar1=b_s[:, m : m + 1],
                        )
                    else:
                        nc.scalar.activation(
                            out=dstT[:, m, c * 512 : (c + 1) * 512], in_=ps,
                            func=AF.Copy,
                        )

        def v_tiles(tiles):
            """V (straight, padded 32-wide blocks; col 16 of each = ones)."""
            for j in tiles:
                ps = psum.tile([P, 512], f32, tag="mix", bufs=2)
                for k in range(NKC):
                    nc.tensor.matmul(
                        ps,
                        lhsT=h1T[:, k, j * P : (j + 1) * P],
                        rhs=wv_s[:, k, :],
                        start=(k == 0), stop=(k == NKC - 1),
                    )
                nc.scalar.copy(Vv[:, j, :], ps)
            ones_cols = Vv.rearrange("p j (h e) -> p j h e", e=HP)[
                :, tiles[0] : tiles[-1] + 1, :, 16:17]
            nc.vector.memset(ones_cols, 1.0)

        # fillers: independent PE work dropped into attention stall points
        fillers = []

        def emit_filler():
            if fillers:
                fillers.pop(0)()

        # ---- attention: unit = (tq-chunk, pack) ----
        def attn_unit(p, cj, fine_norm=False):
            expc = attn.tile([P, NPACK, NT, 512], bf16, tag="expc", bufs=2,
                             name=f"expc{p}_{cj}")
            tiles = list(range(0, min(NT, 4 * cj + 4)))
            # S^T as 32x32 subarray tiles; 2 heads share one 2-bank psum tile.
            # exp evac: heads 0-2 table-exp on ScalarE, head 3 Schraudolph
            # bf16-bits exp on VectorE (DVE also owns mask+normalize).
            for i in tiles:
                off = max(0, P * i - 512 * cj)  # valid start within chunk
                n = 512 - off
                # one psum alloc per head-pair: ring depth 2 pipelines
                # S^T(i+1) against the exp evac of pair (i, h01)
                for q in range(2):
                    sp = psum.tile([P, 2, 512], f32, tag="sps", bufs=2,
                                   name=f"sp{p}_{cj}_{i}_{q}")
                    for e in range(2):
                        hh = 2 * q + e
                        nc.tensor.matmul(
                            sp[:, e, 0:n],
                            lhsT=KT[HP * hh : HP * (hh + 1), p,
                                    i * P : (i + 1) * P],
                            rhs=QT[HP * hh : HP * (hh + 1), p,
                                   512 * cj + off : 512 * cj + off + n],
                            start=True, stop=True,
                            tile_position=(HP * hh, 0),
                        )
                    if q == 0:
                        nc.scalar.activation(
                            out=expc[:, 0:2, i, off : off + n],
                            in_=sp[:, :, 0:n],
                            func=AF.Exp, scale=SCALE,
                        )
                    else:
                        nc.vector.tensor_scalar(
                            out=expc[:, 2:4, i, off : off + n].bitcast(i16),
                            in0=sp[:, :, 0:n],
                            scalar1=EXP_A, scalar2=EXP_B,
                            op0=ALU.mult, op1=ALU.add,
                        )
                if i % 4 == 3:
                    emit_filler()
            # causal mask: the 4 diagonal blocks of this chunk per head
            tri_r = bass.AP(
                tensor=tri_s.tensor, offset=tri_s.offset,
                ap=[list(tri_s.ap[0]), [0, 4], [1, P]],
            )
            for hh in range(NPACK):
                base = expc[:, hh]
                dview = bass.AP(
                    tensor=base.tensor,
                    offset=base.offset + 2048 * cj,
                    ap=[list(base.ap[0]), [512 + P, 4], [1, P]],
                )
                # split the diag-mask multiplies: DVE is fast (bf16 2x) but
                # loaded; GpSimd is slower but otherwise idle — they run in
                # parallel so the per-unit mask wall is ~max of the two
                eng = nc.vector if hh < 2 else nc.gpsimd
                eng.tensor_tensor(
                    out=dview, in0=dview, in1=tri_r, op=ALU.mult,
                )
            # PV accumulation over valid tk tiles
            pv = psum.tile([P, 512], f32, tag="pv", bufs=2, name=f"pv{p}_{cj}")
            last = max(tiles)
            for i in tiles:
                off = max(0, P * i - 512 * cj)
                n = 512 - off
                for hh in range(NPACK):
                    h = 4 * p + hh
                    nc.tensor.matmul(
                        pv[HP * hh : HP * (hh + 1), off : off + n],
                        lhsT=Vv[:, i, HP * h : HP * (h + 1)],
                        rhs=expc[:, hh, i, off : off + n],
                        start=(i == 0), stop=(i == last),
                        tile_position=(0, HP * hh),
                        skip_group_check=True,
                    )
            # normalize: out^T = pv / Z  (Z in partition 16 of each 32-block)
            zbc = work.tile([P, 512], f32, tag="zbc")
            rz = work.tile([P, 512], f32, tag="rz")
            if fine_norm:
                # last unit: pipeline the normalize in 128-col pieces so the
                # projection (which consumes per-128-col tiles) starts early
                for s in range(4):
                    sl = slice(128 * s, 128 * (s + 1))
                    nc.vector.stream_shuffle(zbc[:, sl], pv[:, sl],
                                             mask=[16] * 32)
                    nc.vector.reciprocal_approx_fast(out=rz[:, sl],
                                                     in_=zbc[:, sl])
                    nc.vector.tensor_tensor(
                        out=OUTT[:, p, 512 * cj + 128 * s :
                                 512 * cj + 128 * (s + 1)],
                        in0=pv[:, sl], in1=rz[:, sl], op=ALU.mult,
                    )
            else:
                nc.vector.stream_shuffle(zbc, pv, mask=[16] * 32)
                nc.vector.reciprocal_approx_fast(out=rz, in_=zbc)
                nc.vector.tensor_tensor(
                    out=OUTT[:, p, 512 * cj : 512 * (cj + 1)], in0=pv, in1=rz,
                    op=ALU.mult,
                )

        def proj_tile(j):
            ps = psum.tile([P, C], f32, tag="mix", bufs=2)
            for k in range(NPACK):
                nc.tensor.matmul(
                    ps,
                    lhsT=OUTT[:, k, j * P : (j + 1) * P],
                    rhs=wp_s[:, k, :],
                    start=(k == 0), stop=(k == NPACK - 1),
                )
            nc.vector.tensor_add(out=x1[:, j], in0=ps, in1=xbp[:, j])

        def ffn1_chunk(c):
            for f in range(NT):
                ffn1_tile(f, c)

        def ffn2_tile(j):
            ps = psum.tile([P, C], f32, tag="mix", bufs=2)
            for f in range(NT):
                nc.tensor.matmul(
                    ps,
                    lhsT=HT[:, f, j * P : (j + 1) * P],
                    rhs=w2_s[:, f, :],
                    start=(f == 0), stop=(f == NT - 1),
                )
            outs = work.tile([P, C], f32, tag="outs")
            nc.vector.tensor_add(out=outs, in0=ps, in1=x1[:, j])
            if not b2_zero:
                nc.vector.tensor_add(out=outs, in0=outs, in1=b2t)
            nc.sync.dma_start(
                out=out_d[:, :].rearrange("(t p) c -> p t c", p=P)[:, j], in_=outs
            )

        # ---- schedule ----
        ln_phase(xs, h1T, "ln1a0", [0, 1])
        ln_phase(xs, h1T, "ln1a1", [2, 3])
        qk_chunk(0)
        v_tiles([0, 1, 2, 3])
        ln_phase(xs, h1T, "ln1b", list(range(4, NT)))
        for m in range(NPACK):
            fillers.append(lambda m=m: qk_pack(wq_s, bq_s, QT, m, 1))
        for p in range(NPACK):
            attn_unit(p, 0)
        while fillers:
            fillers.pop(0)()
        for m in range(NPACK):
            qk_pack(wk_s, bk_s, KT, m, 1)
        v_tiles([4, 5, 6, 7])
        for j in range(4):
            proj_tile(j)
        ln_phase(x1, h2T, "ln2_0", list(range(4)))
        for f in range(NT):
            fillers.append(lambda f=f: ffn1_tile(f, 0))
        for p in range(NPACK):
            attn_unit(p, 1, fine_norm=(p == NPACK - 1))
        while fillers:
            fillers.pop(0)()
        for j in range(4):
            ffn2_tile(j)
        for j in range(4, NT):
            proj_tile(j)
        ln_phase(x1, h2T, "ln2_1", list(range(4, NT)))
        ffn1_chunk(1)
        for j in range(4, NT):
            ffn2_tile(j)

        for pool in (psum, work, attn, data, consts):
            pool.release()

    nc.compile()
    return nc


def _prep_inputs(x, Wq, Wk, Wv, Wp, bp, W1, b1, W2, b2, g1, be1, g2, be2):
    """Host-side preprocessing: fold LN affines into the following matmuls,
    pad per-head weights to 32-wide blocks, cast to bf16."""
    f32 = np.float32
    x = np.asarray(x, f32).astype(_BF16)
    Wqf = np.asarray(Wq, f32).reshape(C, C) * np.asarray(g1, f32)[:, None]
    Wkf = np.asarray(Wk, f32).reshape(C, C) * np.asarray(g1, f32)[:, None]
    Wvf = np.asarray(Wv, f32).reshape(C, C) * np.asarray(g1, f32)[:, None]
    bqf = np.asarray(be1, f32) @ np.asarray(Wq, f32).reshape(C, C)
    bkf = np.asarray(be1, f32) @ np.asarray(Wk, f32).reshape(C, C)
    bvf = np.asarray(be1, f32) @ np.asarray(Wv, f32).reshape(C, C)

    def pad_cols(w):
        wp = np.zeros((C, CP), f32)
        for h in range(H):
            wp[:, HP * h : HP * h + D] = w[:, D * h : D * (h + 1)]
        return wp

    def pad_vec(v):
        vp = np.zeros((CP,), f32)
        for h in range(H):
            vp[HP * h : HP * h + D] = v[D * h : D * (h + 1)]
        return vp

    wq_p = pad_cols(Wqf)
    wk_p = pad_cols(Wkf)
    wv_p = pad_cols(Wvf)
    bq_p = pad_vec(bqf)
    bk_p = pad_vec(bkf)
    bv_p = pad_vec(bvf)

    wp_p = np.zeros((CP, C), f32)
    for h in range(H):
        wp_p[HP * h : HP * h + D, :] = np.asarray(Wp, f32)[D * h : D * (h + 1), :]

    W1f = np.asarray(W1, f32) * np.asarray(g2, f32)[:, None]
    b1f = np.asarray(b1, f32) + np.asarray(be2, f32) @ np.asarray(W1, f32)

    shared = {
        "wq": wq_p.astype(_BF16), "wk": wk_p.astype(_BF16),
        "wv": wv_p.astype(_BF16), "wp": wp_p.astype(_BF16),
        "w1": W1f.astype(_BF16), "w2": np.asarray(W2, f32).astype(_BF16),
        "bq": bq_p, "bk": bk_p,
        "bprow": np.asarray(bp, f32), "b1p": b1f,
        "b2row": np.asarray(b2, f32),
    }
    assert not np.any(bv_p), "nonzero V bias not folded on-device (be1 != 0)"
    return x, shared


def kernel(**inputs) -> np.ndarray:
    from concourse import bass_utils

    x, shared = _prep_inputs(**inputs)
    qk_bias = bool(np.any(shared["bq"]) or np.any(shared["bk"]))
    bp_zero = not np.any(shared["bprow"])
    b2_zero = not np.any(shared["b2row"])
    b1_zero = not np.any(shared["b1p"])
    key = ("nc", qk_bias, bp_zero, b2_zero, b1_zero)
    if key not in _cache:
        _cache[key] = _build_program(
            qk_bias=qk_bias, bp_zero=bp_zero, b2_zero=b2_zero, b1_zero=b1_zero)
    nc = _cache[key]

    in_maps = [dict(shared, x=np.ascontiguousarray(x[i])) for i in range(B)]
    res = bass_utils.run_bass_kernel_spmd(nc, in_maps, core_ids=list(range(B)))
    _cache["last_result"] = res
    out = np.stack([r["out"] for r in res.results], axis=0)
    return out.astype(np.float32)



# revision 14
# speedup vs baseline: 1.0961x; 1.0961x over previous
"""Trainium2 Bass kernel for one pre-LN transformer block (B=8, T=1024, C=256,
H=16 heads of size 16, FFN 256->1024->256), data-parallel over batch across 8
NeuronCores (one batch element per core).

Per-core dataflow (matmul operands bf16, accumulation fp32):
  x arrives bf16 per-tile on three DMA queues (sync/scalar/gpsimd) so LN1
    starts ~9us in; LN affines are folded into the QKV/FFN1 weights host-side
  LN1 (straight [T,C]; batched Quake-rsqrt on DVE, apply+evacs on ScalarE
    with per-partition scale/bias) -> PE-transpose -> h1^T [C,T]
  Q^T/K^T in padded head layout [h*32+d, T] (pad rows zero, padded weights)
  V straight [T, h*32+{d,16=ones-col,zeros}] - the ones column makes the PV
    matmul also produce the softmax denominator (scores are tiny: no max pass)
  S^T[tk,tq] = k^T.T @ q^T per head via 32-row-strip matmuls (4 heads share
    the 128-row PE array); one PSUM alloc per head-PAIR so the ring pipelines
    S^T(i+1) against exp(i); exp fused into the PSUM->SBUF evac: heads 0-1
    table-exp on ScalarE, heads 2-3 Schraudolph bf16-bits exp on VectorE;
    causal diag blocks masked by one batched diagonal-strided triangular
    multiply per (pack, head) against a single [128,128] mask read 4x via a
    stride-0 middle AP dim
  PV: out^T[d,tq] accumulated over tk tiles with 32-col-strip matmuls
  normalize via per-head Z row broadcast (stream_shuffle from PSUM) +
    reciprocal_approx_fast + multiply (the last unit runs this in 128-col
    pieces so the projection can start early)
  proj: x1 = x(+bp) + out^T.T @ Wp   (out^T tiles are the stationary operand)
  LN2 -> h2^T -> FFN1 (relu+bias on ScalarE evac) -> FFN2 -> + x1

Scheduling: program order interleaves chunk production with attention so the
PE never waits on a full-phase barrier, and independent GEMMs are dropped
into attention stall points as "fillers" (QK chunk-1 packs into attention
chunk 0, FFN1 chunk-0 tiles into attention chunk 1) to keep the tensor
engine dense while exp evacuations drain.
"""

import os
import sys

for _p in ("/opt/trn_rl_repo", "/root/.axon_site/_ro/trn_rl_repo"):
    if os.path.isdir(_p) and _p not in sys.path:
        sys.path.append(_p)

import numpy as np
import ml_dtypes

# problem shapes (hardcoded per contest rules)
B, T, C, H, D, F = 8, 1024, 256, 16, 16, 1024
P = 128          # partitions
NT = T // P      # 8 T-tiles
HP = 32          # padded per-head stride (Q/K/V/out layouts)
CP = H * HP      # 512 padded channel dim
NPACK = 4        # head packs (4 heads per 128-partition tile)
NKC = C // P     # 2 k-tiles over C
EPS = 1e-5
SCALE = D ** -0.5
MAGIC = 0x5F3759DF
# Schraudolph-style exp to bf16 bits: bf16_bits(exp(SCALE*s)) ~= EXP_A*s + EXP_B
EXP_A = (2 ** 7) * SCALE * 1.4426950408889634
EXP_B = 2 ** 7 * 127 - 5.6

_BF16 = ml_dtypes.bfloat16

_cache = {}


def _build_program(qk_bias=False, bp_zero=False, b2_zero=False, b1_zero=False):
    import concourse.bass as bass
    import concourse.bacc as bacc
    import concourse.tile as tile
    import concourse.mybir as mybir

    dt = mybir.dt
    f32, bf16, i32, i16 = dt.float32, dt.bfloat16, dt.int32, dt.int16
    AF = mybir.ActivationFunctionType
    ALU = mybir.AluOpType

    nc = bacc.Bacc("TRN2", target_bir_lowering=False, debug=False)

    # ---- DRAM I/O ----
    x_d = nc.dram_tensor("x", [T, C], bf16, kind="ExternalInput")
    wq_d = nc.dram_tensor("wq", [C, CP], bf16, kind="ExternalInput")
    wk_d = nc.dram_tensor("wk", [C, CP], bf16, kind="ExternalInput")
    wv_d = nc.dram_tensor("wv", [C, CP], bf16, kind="ExternalInput")
    wp_d = nc.dram_tensor("wp", [CP, C], bf16, kind="ExternalInput")
    w1_d = nc.dram_tensor("w1", [C, F], bf16, kind="ExternalInput")
    w2_d = nc.dram_tensor("w2", [F, C], bf16, kind="ExternalInput")
    bq_d = nc.dram_tensor("bq", [CP], f32, kind="ExternalInput")
    bk_d = nc.dram_tensor("bk", [CP], f32, kind="ExternalInput")
    bp_d = nc.dram_tensor("bprow", [C], f32, kind="ExternalInput")
    b1_d = nc.dram_tensor("b1p", [F], f32, kind="ExternalInput")
    b2_d = nc.dram_tensor("b2row", [C], f32, kind="ExternalInput")
    out_d = nc.dram_tensor("out", [T, C], f32, kind="ExternalOutput")

    ident_np = np.eye(P, dtype=_BF16)
    # S^T diag tile mask: partition = tk local, free = tq local; keep tq >= tk
    tri_np = np.triu(np.ones((P, P), dtype=np.float32)).astype(_BF16)
    ident_d = nc.inline_tensor(ident_np, name="ident")
    tri_d = nc.inline_tensor(tri_np, name="trimask")

    with tile.TileContext(nc) as tc:
        consts = tc.alloc_tile_pool(name="consts", bufs=1)
        data = tc.alloc_tile_pool(name="data", bufs=1)
        attn = tc.alloc_tile_pool(name="attn", bufs=1)
        work = tc.alloc_tile_pool(name="work", bufs=4)
        psum = tc.alloc_tile_pool(name="psum", bufs=1, space="PSUM")

        # ---- persistent SBUF tensors ----
        ident_s = consts.tile([P, P], bf16)
        tri_s = consts.tile([P, P], bf16)
        wq_s = consts.tile([P, NKC, CP], bf16)
        wk_s = consts.tile([P, NKC, CP], bf16)
        wv_s = consts.tile([P, NKC, CP], bf16)
        wp_s = consts.tile([P, NPACK, C], bf16)
        w1_s = consts.tile([P, NKC, F], bf16)
        w2_s = consts.tile([P, NT, C], bf16)
        bq_s = consts.tile([P, NPACK], f32)
        bk_s = consts.tile([P, NPACK], f32)
        b1_s = consts.tile([P, NT], f32)

        xs = data.tile([P, NT, C], bf16)
        xbp = xs if bp_zero else data.tile([P, NT, C], bf16)
        h1T = data.tile([P, NKC, T], bf16)
        QT = data.tile([P, NPACK, T], bf16)
        KT = data.tile([P, NPACK, T], bf16)
        Vv = data.tile([P, NT, CP], bf16)
        OUTT = data.tile([P, NPACK, T], bf16)
        x1 = data.tile([P, NT, C], f32)
        h2T = data.tile([P, NKC, T], bf16)
        HT = data.tile([P, NT, F], bf16)

        # ---- input DMAs: x tiles first on the two fast HWDGE queues
        # (sync/scalar) so LN1 starts ASAP; weights go on the gpsimd SWDGE
        # queue, which also leaves the gpsimd engine free afterwards ----
        x_r = x_d[:, :].rearrange("(j p) c -> p j c", p=P)
        x_engines = [nc.sync, nc.scalar, nc.sync, nc.scalar,
                     nc.sync, nc.scalar, nc.sync, nc.scalar]
        for j in range(NT):
            x_engines[j].dma_start(out=xs[:, j], in_=x_r[:, j])
        nc.gpsimd.dma_start(out=ident_s, in_=ident_d[:, :])
        nc.gpsimd.dma_start(out=wq_s, in_=wq_d[:, :].rearrange("(k p) c -> p k c", p=P))
        nc.gpsimd.dma_start(out=wk_s, in_=wk_d[:, :].rearrange("(k p) c -> p k c", p=P))
        nc.gpsimd.dma_start(out=wv_s, in_=wv_d[:, :].rearrange("(k p) c -> p k c", p=P))
        nc.gpsimd.dma_start(out=tri_s, in_=tri_d[:, :])
        nc.gpsimd.dma_start(out=wp_s, in_=wp_d[:, :].rearrange("(k p) c -> p k c", p=P))
        nc.scalar.dma_start(out=w1_s, in_=w1_d[:, :].rearrange("(k p) c -> p k c", p=P))
        nc.sync.dma_start(out=w2_s, in_=w2_d[:, :].rearrange("(k p) c -> p k c", p=P))
        if not bp_zero:
            nc.scalar.dma_start(
                out=xbp, in_=x_d[:, :].rearrange("(j p) c -> p j c", p=P))
            # add bp (broadcast along partitions) into the residual copy
            bp_b = bass.AP(tensor=bp_d, offset=0, ap=[[0, P], [1, C]])
            bpt = consts.tile([P, C], f32)
            nc.sync.dma_start(out=bpt, in_=bp_b)
            for j in range(NT):
                nc.vector.tensor_add(out=xbp[:, j], in0=xbp[:, j], in1=bpt)
        if qk_bias:
            nc.sync.dma_start(out=bq_s, in_=bq_d[:].rearrange("(m p) -> p m", p=P))
            nc.sync.dma_start(out=bk_s, in_=bk_d[:].rearrange("(m p) -> p m", p=P))
        if not b1_zero:
            nc.sync.dma_start(out=b1_s, in_=b1_d[:].rearrange("(m p) -> p m", p=P))
        if not b2_zero:
            b2t = consts.tile([P, C], f32)
            b2_b = bass.AP(tensor=b2_d, offset=0, ap=[[0, P], [1, C]])
            nc.sync.dma_start(out=b2t, in_=b2_b)

        def ln_phase(src, dst_hT, tag, tiles, sc_stats=False):
            """LayerNorm the given tiles of src [128, 8, 256] f32 and write
            the transposed bf16 result into dst_hT [128, 2, 1024].
            sc_stats: compute Σx/Σx² on ScalarE (accum_out) instead of DVE
            bn_stats — used where the DVE is the exposed critical path."""
            nj = len(tiles)
            mvall = work.tile([P, nj, 2], f32, tag="mvall", name=f"mv_{tag}")
            vpe = work.tile([P, nj], f32, tag="vpe", name=f"vpe_{tag}")
            if sc_stats:
                sx = work.tile([P, nj], f32, tag="sx", name=f"sx_{tag}")
                sxx = work.tile([P, nj], f32, tag="sxx", name=f"sxx_{tag}")
                for jx, j in enumerate(tiles):
                    scr = work.tile([P, C], f32, tag="scr")
                    nc.scalar.activation(
                        out=scr, in_=src[:, j], func=AF.Identity,
                        accum_out=sx[:, jx : jx + 1])
                    scr2 = work.tile([P, C], f32, tag="scr")
                    nc.scalar.activation(
                        out=scr2, in_=src[:, j], func=AF.Square,
                        accum_out=sxx[:, jx : jx + 1])
                nc.vector.tensor_scalar(
                    out=mvall[:, :, 0], in0=sx, scalar1=1.0 / C, scalar2=None,
                    op0=ALU.mult)
                mm = work.tile([P, nj], f32, tag="mm2", name=f"mm2_{tag}")
                nc.vector.tensor_tensor(
                    out=mm, in0=mvall[:, :, 0], in1=mvall[:, :, 0], op=ALU.mult)
                nc.vector.tensor_scalar(
                    out=vpe, in0=sxx, scalar1=1.0 / C, scalar2=EPS,
                    op0=ALU.mult, op1=ALU.add)
                nc.vector.tensor_tensor(
                    out=vpe, in0=vpe, in1=mm, op=ALU.subtract)
            else:
                for jx, j in enumerate(tiles):
                    stats = work.tile([P, 6], f32, tag="stats")
                    nc.vector.bn_stats(out=stats, in_=src[:, j])
                    nc.vector.bn_aggr(out=mvall[:, jx], in_=stats)
                nc.vector.tensor_scalar_add(
                    out=vpe, in0=mvall[:, :, 1], scalar1=EPS)
            # rstd for all tiles: Quake rsqrt + 2 Newton steps (pure DVE)
            sh = work.tile([P, nj], i32, tag="rsq_sh")
            nc.vector.tensor_scalar(
                out=sh, in0=vpe.bitcast(i32), scalar1=1, scalar2=None,
                op0=ALU.logical_shift_right,
            )
            y0 = work.tile([P, nj], i32, tag="rsq_y0")
            nc.vector.tensor_scalar(
                out=y0, in0=sh, scalar1=-1, scalar2=MAGIC,
                op0=ALU.mult, op1=ALU.add,
            )
            y = y0.bitcast(f32)
            rsq = work.tile([P, nj], f32, tag="rsq", name=f"rsq_{tag}")
            tmp = work.tile([P, nj], f32, tag="rsq_tmp")
            for it in range(2):
                nc.vector.tensor_tensor(out=tmp, in0=y, in1=y, op=ALU.mult)
                nc.vector.tensor_tensor(out=tmp, in0=tmp, in1=vpe, op=ALU.mult)
                nc.vector.tensor_scalar(
                    out=tmp, in0=tmp, scalar1=-0.5, scalar2=1.5,
                    op0=ALU.mult, op1=ALU.add,
                )
                nc.vector.tensor_tensor(out=rsq, in0=tmp, in1=y, op=ALU.mult)
                y = rsq
            # bias for the ScalarE apply: -mu*rstd
            nmurs = work.tile([P, nj], f32, tag="nmurs", name=f"nmurs_{tag}")
            nc.vector.tensor_tensor(
                out=nmurs, in0=mvall[:, :, 0], in1=rsq, op=ALU.mult)
            nc.vector.tensor_scalar(
                out=nmurs, in0=nmurs, scalar1=-1.0, scalar2=None, op0=ALU.mult)
            for jx, j in enumerate(tiles):
                # apply on GpSimd (SBUF->SBUF, frees ScalarE for exp/evacs)
                hs = work.tile([P, C], bf16, tag="hstraight")
                nc.gpsimd.tensor_scalar(
                    out=hs, in0=src[:, j],
                    scalar1=rsq[:, jx : jx + 1], scalar2=nmurs[:, jx : jx + 1],
                    op0=ALU.mult, op1=ALU.add,
                )
                tp = psum.tile([P, 2, P], bf16, tag="mix", bufs=2)
                nc.tensor.transpose(tp[:, 0], hs[:, 0:P], ident_s)
                nc.tensor.transpose(tp[:, 1], hs[:, P : 2 * P], ident_s)
                nc.scalar.activation(
                    out=dst_hT[:, :, j * P : (j + 1) * P], in_=tp, func=AF.Copy)

        def qk_pack(w_s, b_s, dstT, m, c):
            ps = psum.tile([P, 512], f32, tag="mix", bufs=2)
            for k in range(NKC):
                nc.tensor.matmul(
                    ps,
                    lhsT=w_s[:, k, m * P : (m + 1) * P],
                    rhs=h1T[:, k, c * 512 : (c + 1) * 512],
                    start=(k == 0), stop=(k == NKC - 1),
                )
            if qk_bias:
                nc.vector.tensor_scalar_add(
                    out=dstT[:, m, c * 512 : (c + 1) * 512], in0=ps,
                    scalar1=b_s[:, m : m + 1],
                )
            else:
                nc.scalar.activation(
                    out=dstT[:, m, c * 512 : (c + 1) * 512], in_=ps,
                    func=AF.Copy,
                )

        def ffn1_tile(f, c):
            ps = psum.tile([P, 512], f32, tag="mix", bufs=2)
            for k in range(NKC):
                nc.tensor.matmul(
                    ps,
                    lhsT=w1_s[:, k, f * P : (f + 1) * P],
                    rhs=h2T[:, k, c * 512 : (c + 1) * 512],
                    start=(k == 0), stop=(k == NKC - 1),
                )
            if b1_zero:
                nc.scalar.activation(
                    out=HT[:, f, c * 512 : (c + 1) * 512], in_=ps, func=AF.Relu)
            else:
                nc.scalar.activation(
                    out=HT[:, f, c * 512 : (c + 1) * 512], in_=ps,
                    func=AF.Relu, bias=b1_s[:, f : f + 1],
                )

        def qk_chunk(c):
            """Q^T / K^T for tq-chunk c (padded layout, bias folded in evac)."""
            for (name, w_s, b_s, dstT) in (
                ("q", wq_s, bq_s, QT), ("k", wk_s, bk_s, KT)):
                for m in range(NPACK):
                    ps = psum.tile([P, 512], f32, tag="mix", bufs=2)
                    for k in range(NKC):
                        nc.tensor.matmul(
                            ps,
                            lhsT=w_s[:, k, m * P : (m + 1) * P],
                            rhs=h1T[:, k, c * 512 : (c + 1) * 512],
                            start=(k == 0), stop=(k == NKC - 1),
                        )
                    if qk_bias:
                        nc.vector.tensor_scalar_add(
                            out=dstT[:, m, c * 512 : (c + 1) * 512], in0=ps,
                            scalar1=b_s[:, m : m + 1],
                        )
                    else:
                        nc.scalar.activation(
                            out=dstT[:, m, c * 512 : (c + 1) * 512], in_=ps,
                            func=AF.Copy,
                        )

        def v_tiles(tiles):
            """V (straight, padded 32-wide blocks; col 16 of each = ones)."""
            for j in tiles:
                ps = psum.tile([P, 512], f32, tag="mix", bufs=2)
                for k in range(NKC):
                    nc.tensor.matmul(
                        ps,
                        lhsT=h1T[:, k, j * P : (j + 1) * P],
                        rhs=wv_s[:, k, :],
                        start=(k == 0), stop=(k == NKC - 1),
                    )
                nc.scalar.copy(Vv[:, j, :], ps)
            ones_cols = Vv.rearrange("p j (h e) -> p j h e", e=HP)[
                :, tiles[0] : tiles[-1] + 1, :, 16:17]
            nc.vector.memset(ones_cols, 1.0)

        # fillers: independent PE work dropped into attention stall points
        fillers = []

        def emit_filler():
            if fillers:
                fillers.pop(0)()

        # ---- attention: unit = (tq-chunk, pack) ----
        def attn_unit(p, cj, fine_norm=False):
            expc = attn.tile([P, NPACK, NT, 512], bf16, tag="expc", bufs=2,
                             name=f"expc{p}_{cj}")
            tiles = list(range(0, min(NT, 4 * cj + 4)))
            # S^T as 32x32 subarray tiles; 2 heads share one 2-bank psum tile.
            # exp evac: heads 0-2 table-exp on ScalarE, head 3 Schraudolph
            # bf16-bits exp on VectorE (DVE also owns mask+normalize).
            for i in tiles:
                off = max(0, P * i - 512 * cj)  # valid start within chunk
                n = 512 - off
                # one psum alloc per head-pair: ring depth 2 pipelines
                # S^T(i+1) against the exp evac of pair (i, h01)
                for q in range(2):
                    sp = psum.tile([P, 2, 512], f32, tag="sps", bufs=2,
                                   name=f"sp{p}_{cj}_{i}_{q}")
                    for e in range(2):
                        hh = 2 * q + e
                        nc.tensor.matmul(
                            sp[:, e, 0:n],
                            lhsT=KT[HP * hh : HP * (hh + 1), p,
                                    i * P : (i + 1) * P],
                            rhs=QT[HP * hh : HP * (hh + 1), p,
                                   512 * cj + off : 512 * cj + off + n],
                            start=True, stop=True,
                            tile_position=(HP * hh, 0),
                        )
                    if q == 0:
                        nc.scalar.activation(
                            out=expc[:, 0:2, i, off : off + n],
                            in_=sp[:, :, 0:n],
                            func=AF.Exp, scale=SCALE,
                        )
                    else:
                        nc.vector.tensor_scalar(
                            out=expc[:, 2:4, i, off : off + n].bitcast(i16),
                            in0=sp[:, :, 0:n],
                            scalar1=EXP_A, scalar2=EXP_B,
                            op0=ALU.mult, op1=ALU.add,
                        )
                if i % 4 == 3:
                    emit_filler()
            # causal mask: the 4 diagonal blocks of this chunk per head
            tri_r = bass.AP(
                tensor=tri_s.tensor, offset=tri_s.offset,
                ap=[list(tri_s.ap[0]), [0, 4], [1, P]],
            )
            for hh in range(NPACK):
                base = expc[:, hh]
                dview = bass.AP(
                    tensor=base.tensor,
                    offset=base.offset + 2048 * cj,
                    ap=[list(base.ap[0]), [512 + P, 4], [1, P]],
                )
                # split the diag-mask multiplies: DVE is fast (bf16 2x) but
                # loaded; GpSimd is slower but otherwise idle — they run in
                # parallel so the per-unit mask wall is ~max of the two
                eng = nc.vector if hh < 2 else nc.gpsimd
                eng.tensor_tensor(
                    out=dview, in0=dview, in1=tri_r, op=ALU.mult,
                )
            # PV accumulation over valid tk tiles
            pv = psum.tile([P, 512], f32, tag="pv", bufs=2, name=f"pv{p}_{cj}")
            last = max(tiles)
            for i in tiles:
                off = max(0, P * i - 512 * cj)
                n = 512 - off
                for hh in range(NPACK):
                    h = 4 * p + hh
                    nc.tensor.matmul(
                        pv[HP * hh : HP * (hh + 1), off : off + n],
                        lhsT=Vv[:, i, HP * h : HP * (h + 1)],
                        rhs=expc[:, hh, i, off : off + n],
                        start=(i == 0), stop=(i == last),
                        tile_position=(0, HP * hh),
                        skip_group_check=True,
                    )
            # normalize: out^T = pv / Z  (Z in partition 16 of each 32-block)
            zbc = work.tile([P, 512], f32, tag="zbc")
            rz = work.tile([P, 512], f32, tag="rz")
            if fine_norm:
                # last unit: pipeline the normalize in 128-col pieces so the
                # projection (which consumes per-128-col tiles) starts early
                for s in range(4):
                    sl = slice(128 * s, 128 * (s + 1))
                    nc.vector.stream_shuffle(zbc[:, sl], pv[:, sl],
                                             mask=[16] * 32)
                    nc.vector.reciprocal_approx_fast(out=rz[:, sl],
                                                     in_=zbc[:, sl])
                    nc.vector.tensor_tensor(
                        out=OUTT[:, p, 512 * cj + 128 * s :
                                 512 * cj + 128 * (s + 1)],
                        in0=pv[:, sl], in1=rz[:, sl], op=ALU.mult,
                    )
            else:
                nc.vector.stream_shuffle(zbc, pv, mask=[16] * 32)
                nc.vector.reciprocal_approx_fast(out=rz, in_=zbc)
                nc.vector.tensor_tensor(
                    out=OUTT[:, p, 512 * cj : 512 * (cj + 1)], in0=pv, in1=rz,
                    op=ALU.mult,
                )

        def proj_tile(j):
            ps = psum.tile([P, C], f32, tag="mix", bufs=2)
            for k in range(NPACK):
                nc.tensor.matmul(
                    ps,
                    lhsT=OUTT[:, k, j * P : (j + 1) * P],
                    rhs=wp_s[:, k, :],
                    start=(k == 0), stop=(k == NPACK - 1),
                )
            nc.vector.tensor_add(out=x1[:, j], in0=ps, in1=xbp[:, j])

        def ffn1_chunk(c):
            for f in range(NT):
                ffn1_tile(f, c)

        def ffn2_tile(j):
            ps = psum.tile([P, C], f32, tag="mix", bufs=2)
            for f in range(NT):
                nc.tensor.matmul(
                    ps,
                    lhsT=HT[:, f, j * P : (j + 1) * P],
                    rhs=w2_s[:, f, :],
                    start=(f == 0), stop=(f == NT - 1),
                )
            outs = work.tile([P, C], f32, tag="outs")
            nc.vector.tensor_add(out=outs, in0=ps, in1=x1[:, j])
            if not b2_zero:
                nc.vector.tensor_add(out=outs, in0=outs, in1=b2t)
            nc.sync.dma_start(
                out=out_d[:, :].rearrange("(t p) c -> p t c", p=P)[:, j], in_=outs
            )

        # ---- schedule ----
        ln_phase(xs, h1T, "ln1a0", [0, 1])
        ln_phase(xs, h1T, "ln1a1", [2, 3])
        qk_chunk(0)
        v_tiles([0, 1, 2, 3])
        ln_phase(xs, h1T, "ln1b", list(range(4, NT)))
        for m in range(NPACK):
            fillers.append(lambda m=m: qk_pack(wq_s, bq_s, QT, m, 1))
        for p in range(NPACK):
            attn_unit(p, 0)
        while fillers:
            fillers.pop(0)()
        for m in range(NPACK):
            qk_pack(wk_s, bk_s, KT, m, 1)
        v_tiles([4, 5, 6, 7])
        for j in range(4):
            proj_tile(j)
        ln_phase(x1, h2T, "ln2_0", list(range(4)))
        for f in range(NT):
            fillers.append(lambda f=f: ffn1_tile(f, 0))
        for p in range(NPACK):
            attn_unit(p, 1, fine_norm=(p == NPACK - 1))
        while fillers:
            fillers.pop(0)()
        for j in range(4):
            ffn2_tile(j)
        for j in range(4, NT):
            proj_tile(j)
        ln_phase(x1, h2T, "ln2_1", list(range(4, NT)))
        ffn1_chunk(1)
        for j in range(4, NT):
            ffn2_tile(j)

        for pool in (psum, work, attn, data, consts):
            pool.release()

    nc.compile()
    return nc


def _prep_inputs(x, Wq, Wk, Wv, Wp, bp, W1, b1, W2, b2, g1, be1, g2, be2):
    """Host-side preprocessing: fold LN affines into the following matmuls,
    pad per-head weights to 32-wide blocks, cast to bf16."""
    f32 = np.float32
    x = np.asarray(x, f32).astype(_BF16)
    Wqf = np.asarray(Wq, f32).reshape(C, C) * np.asarray(g1, f32)[:, None]
    Wkf = np.asarray(Wk, f32).reshape(C, C) * np.asarray(g1, f32)[:, None]
    Wvf = np.asarray(Wv, f32).reshape(C, C) * np.asarray(g1, f32)[:, None]
    bqf = np.asarray(be1, f32) @ np.asarray(Wq, f32).reshape(C, C)
    bkf = np.asarray(be1, f32) @ np.asarray(Wk, f32).reshape(C, C)
    bvf = np.asarray(be1, f32) @ np.asarray(Wv, f32).reshape(C, C)

    def pad_cols(w):
        wp = np.zeros((C, CP), f32)
        for h in range(H):
            wp[:, HP * h : HP * h + D] = w[:, D * h : D * (h + 1)]
        return wp

    def pad_vec(v):
        vp = np.zeros((CP,), f32)
        for h in range(H):
            vp[HP * h : HP * h + D] = v[D * h : D * (h + 1)]
        return vp

    wq_p = pad_cols(Wqf)
    wk_p = pad_cols(Wkf)
    wv_p = pad_cols(Wvf)
    bq_p = pad_vec(bqf)
    bk_p = pad_vec(bkf)
    bv_p = pad_vec(bvf)

    wp_p = np.zeros((CP, C), f32)
    for h in range(H):
        wp_p[HP * h : HP * h + D, :] = np.asarray(Wp, f32)[D * h : D * (h + 1), :]

    W1f = np.asarray(W1, f32) * np.asarray(g2, f32)[:, None]
    b1f = np.asarray(b1, f32) + np.asarray(be2, f32) @ np.asarray(W1, f32)

    shared = {
        "wq": wq_p.astype(_BF16), "wk": wk_p.astype(_BF16),
        "wv": wv_p.astype(_BF16), "wp": wp_p.astype(_BF16),
        "w1": W1f.astype(_BF16), "w2": np.asarray(W2, f32).astype(_BF16),
        "bq": bq_p, "bk": bk_p,
        "bprow": np.asarray(bp, f32), "b1p": b1f,
        "b2row": np.asarray(b2, f32),
    }
    assert not np.any(bv_p), "nonzero V bias not folded on-device (be1 != 0)"
    return x, shared


def kernel(**inputs) -> np.ndarray:
    from concourse import bass_utils

    x, shared = _prep_inputs(**inputs)
    qk_bias = bool(np.any(shared["bq"]) or np.any(shared["bk"]))
    bp_zero = not np.any(shared["bprow"])
    b2_zero = not np.any(shared["b2row"])
    b1_zero = not np.any(shared["b1p"])
    key = ("nc", qk_bias, bp_zero, b2_zero, b1_zero)
    if key not in _cache:
        _cache[key] = _build_program(
            qk_bias=qk_bias, bp_zero=bp_zero, b2_zero=b2_zero, b1_zero=b1_zero)
    nc = _cache[key]

    in_maps = [dict(shared, x=np.ascontiguousarray(x[i])) for i in range(B)]
    res = bass_utils.run_bass_kernel_spmd(nc, in_maps, core_ids=list(range(B)))
    _cache["last_result"] = res
    out = np.stack([r["out"] for r in res.results], axis=0)
    return out.astype(np.float32)



# revision 22
# speedup vs baseline: 1.1074x; 1.0103x over previous
"""Trainium2 Bass kernel for one pre-LN transformer block (B=8, T=1024, C=256,
H=16 heads of size 16, FFN 256->1024->256), data-parallel over batch across 8
NeuronCores (one batch element per core).

Per-core dataflow (matmul operands bf16, accumulation fp32):
  x arrives bf16 per-tile on three DMA queues (sync/scalar/gpsimd) so LN1
    starts ~9us in; LN affines are folded into the QKV/FFN1 weights host-side
  LN1 (straight [T,C]; batched Quake-rsqrt on DVE, apply+evacs on ScalarE
    with per-partition scale/bias) -> PE-transpose -> h1^T [C,T]
  Q^T/K^T in padded head layout [h*32+d, T] (pad rows zero, padded weights)
  V straight [T, h*32+{d,16=ones-col,zeros}] - the ones column makes the PV
    matmul also produce the softmax denominator (scores are tiny: no max pass)
  S^T[tk,tq] = k^T.T @ q^T per head via 32-row-strip matmuls (4 heads share
    the 128-row PE array); one PSUM alloc per head-PAIR so the ring pipelines
    S^T(i+1) against exp(i); exp fused into the PSUM->SBUF evac: heads 0-1
    table-exp on ScalarE, heads 2-3 Schraudolph bf16-bits exp on VectorE;
    causal diag blocks masked by one batched diagonal-strided triangular
    multiply per (pack, head) against a single [128,128] mask read 4x via a
    stride-0 middle AP dim
  PV: out^T[d,tq] accumulated over tk tiles with 32-col-strip matmuls
  normalize via per-head Z row broadcast (stream_shuffle from PSUM) +
    reciprocal_approx_fast + multiply (the last unit runs this in 128-col
    pieces so the projection can start early)
  proj: x1 = x(+bp) + out^T.T @ Wp   (out^T tiles are the stationary operand)
  LN2 -> h2^T -> FFN1 (relu+bias on ScalarE evac) -> FFN2 -> + x1

Scheduling: program order interleaves chunk production with attention so the
PE never waits on a full-phase barrier, and independent GEMMs are dropped
into attention stall points as "fillers" (QK chunk-1 packs into attention
chunk 0, FFN1 chunk-0 tiles into attention chunk 1) to keep the tensor
engine dense while exp evacuations drain.
"""

import os
import sys

for _p in ("/opt/trn_rl_repo", "/root/.axon_site/_ro/trn_rl_repo"):
    if os.path.isdir(_p) and _p not in sys.path:
        sys.path.append(_p)

import numpy as np
import ml_dtypes

# problem shapes (hardcoded per contest rules)
B, T, C, H, D, F = 8, 1024, 256, 16, 16, 1024
P = 128          # partitions
NT = T // P      # 8 T-tiles
HP = 32          # padded per-head stride (Q/K/V/out layouts)
CP = H * HP      # 512 padded channel dim
NPACK = 4        # head packs (4 heads per 128-partition tile)
NKC = C // P     # 2 k-tiles over C
EPS = 1e-5
SCALE = D ** -0.5
MAGIC = 0x5F3759DF
# Schraudolph-style exp to bf16 bits: bf16_bits(exp(SCALE*s)) ~= EXP_A*s + EXP_B
EXP_A = (2 ** 7) * SCALE * 1.4426950408889634
EXP_B = 2 ** 7 * 127 - 5.6

_BF16 = ml_dtypes.bfloat16

_cache = {}


def _build_program(qk_bias=False, bp_zero=False, b2_zero=False, b1_zero=False):
    import concourse.bass as bass
    import concourse.bacc as bacc
    import concourse.tile as tile
    import concourse.mybir as mybir

    dt = mybir.dt
    f32, bf16, i32, i16 = dt.float32, dt.bfloat16, dt.int32, dt.int16
    AF = mybir.ActivationFunctionType
    ALU = mybir.AluOpType

    nc = bacc.Bacc("TRN2", target_bir_lowering=False, debug=False)

    # ---- DRAM I/O ----
    x_d = nc.dram_tensor("x", [T, C], bf16, kind="ExternalInput")
    wq_d = nc.dram_tensor("wq", [C, CP], bf16, kind="ExternalInput")
    wk_d = nc.dram_tensor("wk", [C, CP], bf16, kind="ExternalInput")
    wv_d = nc.dram_tensor("wv", [C, CP], bf16, kind="ExternalInput")
    wp_d = nc.dram_tensor("wp", [CP, C], bf16, kind="ExternalInput")
    w1_d = nc.dram_tensor("w1", [C, F], bf16, kind="ExternalInput")
    w2_d = nc.dram_tensor("w2", [F, C], bf16, kind="ExternalInput")
    bq_d = nc.dram_tensor("bq", [CP], f32, kind="ExternalInput")
    bk_d = nc.dram_tensor("bk", [CP], f32, kind="ExternalInput")
    bp_d = nc.dram_tensor("bprow", [C], f32, kind="ExternalInput")
    b1_d = nc.dram_tensor("b1p", [F], f32, kind="ExternalInput")
    b2_d = nc.dram_tensor("b2row", [C], f32, kind="ExternalInput")
    out_d = nc.dram_tensor("out", [T, C], f32, kind="ExternalOutput")

    ident_np = np.eye(P, dtype=_BF16)
    # S^T diag tile mask: partition = tk local, free = tq local; keep tq >= tk
    tri_np = np.triu(np.ones((P, P), dtype=np.float32)).astype(_BF16)
    ident_d = nc.inline_tensor(ident_np, name="ident")
    tri_d = nc.inline_tensor(tri_np, name="trimask")

    with tile.TileContext(nc) as tc:
        consts = tc.alloc_tile_pool(name="consts", bufs=1)
        data = tc.alloc_tile_pool(name="data", bufs=1)
        attn = tc.alloc_tile_pool(name="attn", bufs=1)
        work = tc.alloc_tile_pool(name="work", bufs=4)
        psum = tc.alloc_tile_pool(name="psum", bufs=1, space="PSUM")

        # ---- persistent SBUF tensors ----
        ident_s = consts.tile([P, P], bf16)
        tri_s = consts.tile([P, P], bf16)
        wq_s = consts.tile([P, NKC, CP], bf16)
        wk_s = consts.tile([P, NKC, CP], bf16)
        wv_s = consts.tile([P, NKC, CP], bf16)
        wp_s = consts.tile([P, NPACK, C], bf16)
        w1_s = consts.tile([P, NKC, F], bf16)
        w2_s = consts.tile([P, NT, C], bf16)
        bq_s = consts.tile([P, NPACK], f32)
        bk_s = consts.tile([P, NPACK], f32)
        b1_s = consts.tile([P, NT], f32)

        xs = data.tile([P, NT, C], bf16)
        xbp = xs if bp_zero else data.tile([P, NT, C], bf16)
        h1T = data.tile([P, NKC, T], bf16)
        QT = data.tile([P, NPACK, T], bf16)
        KT = data.tile([P, NPACK, T], bf16)
        Vv = data.tile([P, NT, CP], bf16)
        OUTT = data.tile([P, NPACK, T], bf16)
        x1 = data.tile([P, NT, C], f32)
        h2T = data.tile([P, NKC, T], bf16)
        HT = data.tile([P, NT, F], bf16)

        # ---- input DMAs: x tiles first on the two fast HWDGE queues
        # (sync/scalar) so LN1 starts ASAP; weights go on the gpsimd SWDGE
        # queue, which also leaves the gpsimd engine free afterwards ----
        x_r = x_d[:, :].rearrange("(j p) c -> p j c", p=P)
        x_engines = [nc.sync, nc.scalar, nc.sync, nc.scalar,
                     nc.sync, nc.scalar, nc.sync, nc.scalar]
        for j in range(NT):
            x_engines[j].dma_start(out=xs[:, j], in_=x_r[:, j])
        nc.gpsimd.dma_start(out=ident_s, in_=ident_d[:, :])
        nc.gpsimd.dma_start(out=wq_s, in_=wq_d[:, :].rearrange("(k p) c -> p k c", p=P))
        nc.gpsimd.dma_start(out=wk_s, in_=wk_d[:, :].rearrange("(k p) c -> p k c", p=P))
        nc.gpsimd.dma_start(out=wv_s, in_=wv_d[:, :].rearrange("(k p) c -> p k c", p=P))
        nc.gpsimd.dma_start(out=tri_s, in_=tri_d[:, :])
        nc.gpsimd.dma_start(out=wp_s, in_=wp_d[:, :].rearrange("(k p) c -> p k c", p=P))
        nc.scalar.dma_start(out=w1_s, in_=w1_d[:, :].rearrange("(k p) c -> p k c", p=P))
        nc.sync.dma_start(out=w2_s, in_=w2_d[:, :].rearrange("(k p) c -> p k c", p=P))
        if not bp_zero:
            nc.scalar.dma_start(
                out=xbp, in_=x_d[:, :].rearrange("(j p) c -> p j c", p=P))
            # add bp (broadcast along partitions) into the residual copy
            bp_b = bass.AP(tensor=bp_d, offset=0, ap=[[0, P], [1, C]])
            bpt = consts.tile([P, C], f32)
            nc.sync.dma_start(out=bpt, in_=bp_b)
            for j in range(NT):
                nc.vector.tensor_add(out=xbp[:, j], in0=xbp[:, j], in1=bpt)
        if qk_bias:
            nc.sync.dma_start(out=bq_s, in_=bq_d[:].rearrange("(m p) -> p m", p=P))
            nc.sync.dma_start(out=bk_s, in_=bk_d[:].rearrange("(m p) -> p m", p=P))
        if not b1_zero:
            nc.sync.dma_start(out=b1_s, in_=b1_d[:].rearrange("(m p) -> p m", p=P))
        if not b2_zero:
            b2t = consts.tile([P, C], f32)
            b2_b = bass.AP(tensor=b2_d, offset=0, ap=[[0, P], [1, C]])
            nc.sync.dma_start(out=b2t, in_=b2_b)

        def ln_phase(src, dst_hT, tag, tiles, sc_stats=False):
            """LayerNorm the given tiles of src [128, 8, 256] f32 and write
            the transposed bf16 result into dst_hT [128, 2, 1024].
            sc_stats: compute Σx/Σx² on ScalarE (accum_out) instead of DVE
            bn_stats — used where the DVE is the exposed critical path."""
            nj = len(tiles)
            mvall = work.tile([P, nj, 2], f32, tag="mvall", name=f"mv_{tag}")
            vpe = work.tile([P, nj], f32, tag="vpe", name=f"vpe_{tag}")
            if sc_stats:
                sx = work.tile([P, nj], f32, tag="sx", name=f"sx_{tag}")
                sxx = work.tile([P, nj], f32, tag="sxx", name=f"sxx_{tag}")
                for jx, j in enumerate(tiles):
                    scr = work.tile([P, C], f32, tag="scr")
                    nc.scalar.activation(
                        out=scr, in_=src[:, j], func=AF.Identity,
                        accum_out=sx[:, jx : jx + 1])
                    scr2 = work.tile([P, C], f32, tag="scr")
                    nc.scalar.activation(
                        out=scr2, in_=src[:, j], func=AF.Square,
                        accum_out=sxx[:, jx : jx + 1])
                nc.vector.tensor_scalar(
                    out=mvall[:, :, 0], in0=sx, scalar1=1.0 / C, scalar2=None,
                    op0=ALU.mult)
                mm = work.tile([P, nj], f32, tag="mm2", name=f"mm2_{tag}")
                nc.vector.tensor_tensor(
                    out=mm, in0=mvall[:, :, 0], in1=mvall[:, :, 0], op=ALU.mult)
                nc.vector.tensor_scalar(
                    out=vpe, in0=sxx, scalar1=1.0 / C, scalar2=EPS,
                    op0=ALU.mult, op1=ALU.add)
                nc.vector.tensor_tensor(
                    out=vpe, in0=vpe, in1=mm, op=ALU.subtract)
            else:
                for jx, j in enumerate(tiles):
                    stats = work.tile([P, 6], f32, tag="stats")
                    nc.vector.bn_stats(out=stats, in_=src[:, j])
                    nc.vector.bn_aggr(out=mvall[:, jx], in_=stats)
                nc.vector.tensor_scalar_add(
                    out=vpe, in0=mvall[:, :, 1], scalar1=EPS)
            # rstd for all tiles: Quake rsqrt + 2 Newton steps (pure DVE)
            sh = work.tile([P, nj], i32, tag="rsq_sh")
            nc.vector.tensor_scalar(
                out=sh, in0=vpe.bitcast(i32), scalar1=1, scalar2=None,
                op0=ALU.logical_shift_right,
            )
            y0 = work.tile([P, nj], i32, tag="rsq_y0")
            nc.vector.tensor_scalar(
                out=y0, in0=sh, scalar1=-1, scalar2=MAGIC,
                op0=ALU.mult, op1=ALU.add,
            )
            y = y0.bitcast(f32)
            rsq = work.tile([P, nj], f32, tag="rsq", name=f"rsq_{tag}")
            tmp = work.tile([P, nj], f32, tag="rsq_tmp")
            for it in range(2):
                nc.vector.tensor_tensor(out=tmp, in0=y, in1=y, op=ALU.mult)
                nc.vector.tensor_tensor(out=tmp, in0=tmp, in1=vpe, op=ALU.mult)
                nc.vector.tensor_scalar(
                    out=tmp, in0=tmp, scalar1=-0.5, scalar2=1.5,
                    op0=ALU.mult, op1=ALU.add,
                )
                nc.vector.tensor_tensor(out=rsq, in0=tmp, in1=y, op=ALU.mult)
                y = rsq
            # bias for the ScalarE apply: -mu*rstd
            nmurs = work.tile([P, nj], f32, tag="nmurs", name=f"nmurs_{tag}")
            nc.vector.tensor_tensor(
                out=nmurs, in0=mvall[:, :, 0], in1=rsq, op=ALU.mult)
            nc.vector.tensor_scalar(
                out=nmurs, in0=nmurs, scalar1=-1.0, scalar2=None, op0=ALU.mult)
            for jx, j in enumerate(tiles):
                hs = work.tile([P, C], bf16, tag="hstraight")
                nc.scalar.activation(
                    out=hs, in_=src[:, j], func=AF.Identity,
                    scale=rsq[:, jx : jx + 1], bias=nmurs[:, jx : jx + 1],
                )
                tp = psum.tile([P, 2, P], bf16, tag="mix", bufs=2)
                nc.tensor.transpose(tp[:, 0], hs[:, 0:P], ident_s)
                nc.tensor.transpose(tp[:, 1], hs[:, P : 2 * P], ident_s)
                nc.scalar.activation(
                    out=dst_hT[:, :, j * P : (j + 1) * P], in_=tp, func=AF.Copy)

        def qk_pack(w_s, b_s, dstT, m, c):
            ps = psum.tile([P, 512], f32, tag="mix", bufs=2)
            for k in range(NKC):
                nc.tensor.matmul(
                    ps,
                    lhsT=w_s[:, k, m * P : (m + 1) * P],
                    rhs=h1T[:, k, c * 512 : (c + 1) * 512],
                    start=(k == 0), stop=(k == NKC - 1),
                )
            if qk_bias:
                nc.vector.tensor_scalar_add(
                    out=dstT[:, m, c * 512 : (c + 1) * 512], in0=ps,
                    scalar1=b_s[:, m : m + 1],
                )
            else:
                nc.scalar.activation(
                    out=dstT[:, m, c * 512 : (c + 1) * 512], in_=ps,
                    func=AF.Copy,
                )

        def ffn1_tile(f, c):
            ps = psum.tile([P, 512], f32, tag="mix", bufs=2)
            for k in range(NKC):
                nc.tensor.matmul(
                    ps,
                    lhsT=w1_s[:, k, f * P : (f + 1) * P],
                    rhs=h2T[:, k, c * 512 : (c + 1) * 512],
                    start=(k == 0), stop=(k == NKC - 1),
                )
            if b1_zero:
                nc.scalar.activation(
                    out=HT[:, f, c * 512 : (c + 1) * 512], in_=ps, func=AF.Relu)
            else:
                nc.scalar.activation(
                    out=HT[:, f, c * 512 : (c + 1) * 512], in_=ps,
                    func=AF.Relu, bias=b1_s[:, f : f + 1],
                )

        def qk_chunk(c):
            """Q^T / K^T for tq-chunk c (padded layout, bias folded in evac).
            During chunk 0 the DVE is idle, so alternate the PSUM evacs
            between ScalarE and DVE to halve the evac critical path."""
            for ni, (name, w_s, b_s, dstT) in enumerate((
                ("q", wq_s, bq_s, QT), ("k", wk_s, bk_s, KT))):
                for m in range(NPACK):
                    ps = psum.tile([P, 512], f32, tag="mix", bufs=2)
                    for k in range(NKC):
                        nc.tensor.matmul(
                            ps,
                            lhsT=w_s[:, k, m * P : (m + 1) * P],
                            rhs=h1T[:, k, c * 512 : (c + 1) * 512],
                            start=(k == 0), stop=(k == NKC - 1),
                        )
                    dst = dstT[:, m, c * 512 : (c + 1) * 512]
                    if qk_bias:
                        nc.vector.tensor_scalar_add(
                            out=dst, in0=ps, scalar1=b_s[:, m : m + 1])
                    elif c == 0 and (ni + m) % 2 == 0:
                        nc.vector.tensor_copy(out=dst, in_=ps)
                    else:
                        nc.scalar.activation(out=dst, in_=ps, func=AF.Copy)

        def v_tiles(tiles, dve_evac=False):
            """V (straight, padded 32-wide blocks; col 16 of each = ones)."""
            for j in tiles:
                ps = psum.tile([P, 512], f32, tag="mix", bufs=2)
                for k in range(NKC):
                    nc.tensor.matmul(
                        ps,
                        lhsT=h1T[:, k, j * P : (j + 1) * P],
                        rhs=wv_s[:, k, :],
                        start=(k == 0), stop=(k == NKC - 1),
                    )
                if dve_evac and j % 2 == 0:
                    nc.vector.tensor_copy(out=Vv[:, j, :], in_=ps)
                else:
                    nc.scalar.copy(Vv[:, j, :], ps)
            ones_cols = Vv.rearrange("p j (h e) -> p j h e", e=HP)[
                :, tiles[0] : tiles[-1] + 1, :, 16:17]
            nc.vector.memset(ones_cols, 1.0)

        # fillers: independent PE work dropped into attention stall points
        fillers = []

        def emit_filler():
            if fillers:
                fillers.pop(0)()

        # ---- attention: unit = (tq-chunk, pack) ----
        def attn_unit(p, cj, fine_norm=False):
            expc = attn.tile([P, NPACK, NT, 512], bf16, tag="expc", bufs=2,
                             name=f"expc{p}_{cj}")
            tiles = list(range(0, min(NT, 4 * cj + 4)))
            # S^T as 32x32 subarray tiles; 2 heads share one 2-bank psum tile.
            # exp evac: heads 0-2 table-exp on ScalarE, head 3 Schraudolph
            # bf16-bits exp on VectorE (DVE also owns mask+normalize).
            for i in tiles:
                off = max(0, P * i - 512 * cj)  # valid start within chunk
                n = 512 - off
                # one psum alloc per head-pair: ring depth 2 pipelines
                # S^T(i+1) against the exp evac of pair (i, h01)
                for q in range(2):
                    sp = psum.tile([P, 2, 512], f32, tag="sps", bufs=2,
                                   name=f"sp{p}_{cj}_{i}_{q}")
                    for e in range(2):
                        hh = 2 * q + e
                        nc.tensor.matmul(
                            sp[:, e, 0:n],
                            lhsT=KT[HP * hh : HP * (hh + 1), p,
                                    i * P : (i + 1) * P],
                            rhs=QT[HP * hh : HP * (hh + 1), p,
                                   512 * cj + off : 512 * cj + off + n],
                            start=True, stop=True,
                            tile_position=(HP * hh, 0),
                        )
                    if q == 0 or i % 4 == 3:
                        # ScalarE table-exp (every 4th tile both pairs go to
                        # ScalarE: ~56/44 split matches the engines' other load)
                        nc.scalar.activation(
                            out=expc[:, 2 * q : 2 * q + 2, i, off : off + n],
                            in_=sp[:, :, 0:n],
                            func=AF.Exp, scale=SCALE,
                        )
                    else:
                        nc.vector.tensor_scalar(
                            out=expc[:, 2:4, i, off : off + n].bitcast(i16),
                            in0=sp[:, :, 0:n],
                            scalar1=EXP_A, scalar2=EXP_B,
                            op0=ALU.mult, op1=ALU.add,
                        )
                if i % 4 == 3:
                    emit_filler()
            # causal mask: all 16 diagonal blocks (4 heads x 4 tiles) of this
            # chunk in ONE bf16-2x DVE op via a 4D strided view
            tri_r = bass.AP(
                tensor=tri_s.tensor, offset=tri_s.offset,
                ap=[list(tri_s.ap[0]), [0, NPACK], [0, 4], [1, P]],
            )
            dview = bass.AP(
                tensor=expc.tensor,
                offset=expc.offset + 2048 * cj,
                ap=[list(expc.ap[0]), [NT * 512, NPACK], [512 + P, 4], [1, P]],
            )
            nc.vector.tensor_tensor(
                out=dview, in0=dview, in1=tri_r, op=ALU.mult,
            )
            # PV accumulation over valid tk tiles
            pv = psum.tile([P, 512], f32, tag="pv", bufs=2, name=f"pv{p}_{cj}")
            last = max(tiles)
            for i in tiles:
                off = max(0, P * i - 512 * cj)
                n = 512 - off
                for hh in range(NPACK):
                    h = 4 * p + hh
                    nc.tensor.matmul(
                        pv[HP * hh : HP * (hh + 1), off : off + n],
                        lhsT=Vv[:, i, HP * h : HP * (h + 1)],
                        rhs=expc[:, hh, i, off : off + n],
                        start=(i == 0), stop=(i == last),
                        tile_position=(0, HP * hh),
                        skip_group_check=True,
                    )
            # normalize: out^T = pv / Z  (Z in partition 16 of each 32-block)
            zbc = work.tile([P, 512], f32, tag="zbc")
            rz = work.tile([P, 512], f32, tag="rz")
            if fine_norm:
                # last unit: pipeline the normalize in 128-col pieces so the
                # projection (which consumes per-128-col tiles) starts early
                for s in range(4):
                    sl = slice(128 * s, 128 * (s + 1))
                    nc.vector.stream_shuffle(zbc[:, sl], pv[:, sl],
                                             mask=[16] * 32)
                    nc.vector.reciprocal_approx_fast(out=rz[:, sl],
                                                     in_=zbc[:, sl])
                    nc.vector.tensor_tensor(
                        out=OUTT[:, p, 512 * cj + 128 * s :
                                 512 * cj + 128 * (s + 1)],
                        in0=pv[:, sl], in1=rz[:, sl], op=ALU.mult,
                    )
            else:
                nc.vector.stream_shuffle(zbc, pv, mask=[16] * 32)
                nc.vector.reciprocal_approx_fast(out=rz, in_=zbc)
                nc.vector.tensor_tensor(
                    out=OUTT[:, p, 512 * cj : 512 * (cj + 1)], in0=pv, in1=rz,
                    op=ALU.mult,
                )

        def proj_tile(j):
            ps = psum.tile([P, C], f32, tag="mix", bufs=2)
            for k in range(NPACK):
                nc.tensor.matmul(
                    ps,
                    lhsT=OUTT[:, k, j * P : (j + 1) * P],
                    rhs=wp_s[:, k, :],
                    start=(k == 0), stop=(k == NPACK - 1),
                )
            nc.vector.tensor_add(out=x1[:, j], in0=ps, in1=xbp[:, j])

        def ffn1_chunk(c):
            for f in range(NT):
                ffn1_tile(f, c)

        def ffn2_tile(j):
            ps = psum.tile([P, C], f32, tag="mix", bufs=2)
            for f in range(NT):
                nc.tensor.matmul(
                    ps,
                    lhsT=HT[:, f, j * P : (j + 1) * P],
                    rhs=w2_s[:, f, :],
                    start=(f == 0), stop=(f == NT - 1),
                )
            outs = work.tile([P, C], f32, tag="outs")
            nc.vector.tensor_add(out=outs, in0=ps, in1=x1[:, j])
            if not b2_zero:
                nc.vector.tensor_add(out=outs, in0=outs, in1=b2t)
            nc.sync.dma_start(
                out=out_d[:, :].rearrange("(t p) c -> p t c", p=P)[:, j], in_=outs
            )

        # ---- schedule ----
        ln_phase(xs, h1T, "ln1a0", [0, 1])
        ln_phase(xs, h1T, "ln1a1", [2, 3])
        qk_chunk(0)
        v_tiles([0, 1, 2, 3], dve_evac=True)
        ln_phase(xs, h1T, "ln1b", list(range(4, NT)))
        for m in range(NPACK):
            fillers.append(lambda m=m: qk_pack(wq_s, bq_s, QT, m, 1))
        for p in range(NPACK):
            attn_unit(p, 0)
        while fillers:
            fillers.pop(0)()
        for m in range(NPACK):
            qk_pack(wk_s, bk_s, KT, m, 1)
        v_tiles([4, 5, 6, 7])
        for j in range(4):
            proj_tile(j)
        ln_phase(x1, h2T, "ln2_0", list(range(4)), sc_stats=True)
        for f in range(NT):
            fillers.append(lambda f=f: ffn1_tile(f, 0))
        for p in range(NPACK):
            attn_unit(p, 1, fine_norm=(p == NPACK - 1))
        while fillers:
            fillers.pop(0)()
        for j in range(4):
            ffn2_tile(j)
        for j in range(4, NT):
            proj_tile(j)
        ln_phase(x1, h2T, "ln2_1", list(range(4, NT)), sc_stats=True)
        ffn1_chunk(1)
        for j in range(4, NT):
            ffn2_tile(j)

        for pool in (psum, work, attn, data, consts):
            pool.release()

    nc.compile()
    return nc


def _prep_inputs(x, Wq, Wk, Wv, Wp, bp, W1, b1, W2, b2, g1, be1, g2, be2):
    """Host-side preprocessing: fold LN affines into the following matmuls,
    pad per-head weights to 32-wide blocks, cast to bf16."""
    f32 = np.float32
    x = np.asarray(x, f32).astype(_BF16)
    Wqf = np.asarray(Wq, f32).reshape(C, C) * np.asarray(g1, f32)[:, None]
    Wkf = np.asarray(Wk, f32).reshape(C, C) * np.asarray(g1, f32)[:, None]
    Wvf = np.asarray(Wv, f32).reshape(C, C) * np.asarray(g1, f32)[:, None]
    bqf = np.asarray(be1, f32) @ np.asarray(Wq, f32).reshape(C, C)
    bkf = np.asarray(be1, f32) @ np.asarray(Wk, f32).reshape(C, C)
    bvf = np.asarray(be1, f32) @ np.asarray(Wv, f32).reshape(C, C)

    def pad_cols(w):
        wp = np.zeros((C, CP), f32)
        for h in range(H):
            wp[:, HP * h : HP * h + D] = w[:, D * h : D * (h + 1)]
        return wp

    def pad_vec(v):
        vp = np.zeros((CP,), f32)
        for h in range(H):
            vp[HP * h : HP * h + D] = v[D * h : D * (h + 1)]
        return vp

    wq_p = pad_cols(Wqf)
    wk_p = pad_cols(Wkf)
    wv_p = pad_cols(Wvf)
    bq_p = pad_vec(bqf)
    bk_p = pad_vec(bkf)
    bv_p = pad_vec(bvf)

    wp_p = np.zeros((CP, C), f32)
    for h in range(H):
        wp_p[HP * h : HP * h + D, :] = np.asarray(Wp, f32)[D * h : D * (h + 1), :]

    W1f = np.asarray(W1, f32) * np.asarray(g2, f32)[:, None]
    b1f = np.asarray(b1, f32) + np.asarray(be2, f32) @ np.asarray(W1, f32)

    shared = {
        "wq": wq_p.astype(_BF16), "wk": wk_p.astype(_BF16),
        "wv": wv_p.astype(_BF16), "wp": wp_p.astype(_BF16),
        "w1": W1f.astype(_BF16), "w2": np.asarray(W2, f32).astype(_BF16),
        "bq": bq_p, "bk": bk_p,
        "bprow": np.asarray(bp, f32), "b1p": b1f,
        "b2row": np.asarray(b2, f32),
    }
    assert not np.any(bv_p), "nonzero V bias not folded on-device (be1 != 0)"
    return x, shared


def kernel(**inputs) -> np.ndarray:
    from concourse import bass_utils

    x, shared = _prep_inputs(**inputs)
    qk_bias = bool(np.any(shared["bq"]) or np.any(shared["bk"]))
    bp_zero = not np.any(shared["bprow"])
    b2_zero = not np.any(shared["b2row"])
    b1_zero = not np.any(shared["b1p"])
    key = ("nc", qk_bias, bp_zero, b2_zero, b1_zero)
    if key not in _cache:
        _cache[key] = _build_program(
            qk_bias=qk_bias, bp_zero=bp_zero, b2_zero=b2_zero, b1_zero=b1_zero)
    nc = _cache[key]

    in_maps = [dict(shared, x=np.ascontiguousarray(x[i])) for i in range(B)]
    res = bass_utils.run_bass_kernel_spmd(nc, in_maps, core_ids=list(range(B)))
    _cache["last_result"] = res
    out = np.stack([r["out"] for r in res.results], axis=0)
    return out.astype(np.float32)



# revision 30
# speedup vs baseline: 1.1170x; 1.0087x over previous
"""Trainium2 Bass kernel for one pre-LN transformer block (B=8, T=1024, C=256,
H=16 heads of size 16, FFN 256->1024->256), data-parallel over batch across 8
NeuronCores (one batch element per core).

Per-core dataflow (matmul operands bf16, accumulation fp32):
  x arrives bf16 per-tile on three DMA queues (sync/scalar/gpsimd) so LN1
    starts ~9us in; LN affines are folded into the QKV/FFN1 weights host-side
  LN1 (straight [T,C]; batched Quake-rsqrt on DVE, apply+evacs on ScalarE
    with per-partition scale/bias) -> PE-transpose -> h1^T [C,T]
  Q^T/K^T in padded head layout [h*32+d, T] (pad rows zero, padded weights)
  V straight [T, h*32+{d,16=ones-col,zeros}] - the ones column makes the PV
    matmul also produce the softmax denominator (scores are tiny: no max pass)
  S^T[tk,tq] = k^T.T @ q^T per head via 32-row-strip matmuls (4 heads share
    the 128-row PE array); one PSUM alloc per head-PAIR so the ring pipelines
    S^T(i+1) against exp(i); exp fused into the PSUM->SBUF evac: heads 0-1
    table-exp on ScalarE, heads 2-3 Schraudolph bf16-bits exp on VectorE;
    causal diag blocks masked by one batched diagonal-strided triangular
    multiply per (pack, head) against a single [128,128] mask read 4x via a
    stride-0 middle AP dim
  PV: out^T[d,tq] accumulated over tk tiles with 32-col-strip matmuls
  normalize via per-head Z row broadcast (stream_shuffle from PSUM) +
    reciprocal_approx_fast + multiply (the last unit runs this in 128-col
    pieces so the projection can start early)
  proj: x1 = x(+bp) + out^T.T @ Wp   (out^T tiles are the stationary operand)
  LN2 -> h2^T -> FFN1 (relu+bias on ScalarE evac) -> FFN2 -> + x1

Scheduling: program order interleaves chunk production with attention so the
PE never waits on a full-phase barrier, and independent GEMMs are dropped
into attention stall points as "fillers" (QK chunk-1 packs into attention
chunk 0, FFN1 chunk-0 tiles into attention chunk 1) to keep the tensor
engine dense while exp evacuations drain.
"""

import os
import sys

for _p in ("/opt/trn_rl_repo", "/root/.axon_site/_ro/trn_rl_repo"):
    if os.path.isdir(_p) and _p not in sys.path:
        sys.path.append(_p)

import numpy as np
import ml_dtypes

# problem shapes (hardcoded per contest rules)
B, T, C, H, D, F = 8, 1024, 256, 16, 16, 1024
P = 128          # partitions
NT = T // P      # 8 T-tiles
HP = 32          # padded per-head stride (Q/K/V/out layouts)
CP = H * HP      # 512 padded channel dim
NPACK = 4        # head packs (4 heads per 128-partition tile)
NKC = C // P     # 2 k-tiles over C
EPS = 1e-5
SCALE = D ** -0.5
MAGIC = 0x5F3759DF
# Schraudolph-style exp to bf16 bits: bf16_bits(exp(SCALE*s)) ~= EXP_A*s + EXP_B
EXP_A = (2 ** 7) * SCALE * 1.4426950408889634
EXP_B = 2 ** 7 * 127 - 5.6

_BF16 = ml_dtypes.bfloat16

_cache = {}


def _build_program(qk_bias=False, bp_zero=False, b2_zero=False, b1_zero=False):
    import concourse.bass as bass
    import concourse.bacc as bacc
    import concourse.tile as tile
    import concourse.mybir as mybir

    dt = mybir.dt
    f32, bf16, i32, i16 = dt.float32, dt.bfloat16, dt.int32, dt.int16
    AF = mybir.ActivationFunctionType
    ALU = mybir.AluOpType

    nc = bacc.Bacc("TRN2", target_bir_lowering=False, debug=False)

    # ---- DRAM I/O ----
    x_d = nc.dram_tensor("x", [T, C], bf16, kind="ExternalInput")
    wq_d = nc.dram_tensor("wq", [C, CP], bf16, kind="ExternalInput")
    wk_d = nc.dram_tensor("wk", [C, CP], bf16, kind="ExternalInput")
    wv_d = nc.dram_tensor("wv", [C, CP], bf16, kind="ExternalInput")
    wp_d = nc.dram_tensor("wp", [CP, C], bf16, kind="ExternalInput")
    w1_d = nc.dram_tensor("w1", [C, F], bf16, kind="ExternalInput")
    w2_d = nc.dram_tensor("w2", [F, C], bf16, kind="ExternalInput")
    bq_d = nc.dram_tensor("bq", [CP], f32, kind="ExternalInput")
    bk_d = nc.dram_tensor("bk", [CP], f32, kind="ExternalInput")
    bp_d = nc.dram_tensor("bprow", [C], f32, kind="ExternalInput")
    b1_d = nc.dram_tensor("b1p", [F], f32, kind="ExternalInput")
    b2_d = nc.dram_tensor("b2row", [C], f32, kind="ExternalInput")
    out_d = nc.dram_tensor("out", [T, C], bf16, kind="ExternalOutput")

    ident_np = np.eye(P, dtype=_BF16)
    # S^T diag tile mask: partition = tk local, free = tq local; keep tq >= tk
    tri_np = np.triu(np.ones((P, P), dtype=np.float32)).astype(_BF16)
    ident_d = nc.inline_tensor(ident_np, name="ident")
    tri_d = nc.inline_tensor(tri_np, name="trimask")

    with tile.TileContext(nc) as tc:
        consts = tc.alloc_tile_pool(name="consts", bufs=1)
        data = tc.alloc_tile_pool(name="data", bufs=1)
        attn = tc.alloc_tile_pool(name="attn", bufs=1)
        work = tc.alloc_tile_pool(name="work", bufs=4)
        psum = tc.alloc_tile_pool(name="psum", bufs=1, space="PSUM")

        # ---- persistent SBUF tensors ----
        ident_s = consts.tile([P, P], bf16)
        tri_s = consts.tile([P, P], bf16)
        wq_s = consts.tile([P, NKC, CP], bf16)
        wk_s = consts.tile([P, NKC, CP], bf16)
        wv_s = consts.tile([P, NKC, CP], bf16)
        wp_s = consts.tile([P, NPACK, C], bf16)
        w1_s = consts.tile([P, NKC, F], bf16)
        w2_s = consts.tile([P, NT, C], bf16)
        bq_s = consts.tile([P, NPACK], f32)
        bk_s = consts.tile([P, NPACK], f32)
        b1_s = consts.tile([P, NT], f32)

        xs = data.tile([P, NT, C], bf16)
        xbp = xs if bp_zero else data.tile([P, NT, C], bf16)
        h1T = data.tile([P, NKC, T], bf16)
        QT = data.tile([P, NPACK, T], bf16)
        KT = data.tile([P, NPACK, T], bf16)
        Vv = data.tile([P, NT, CP], bf16)
        OUTT = data.tile([P, NPACK, T], bf16)
        x1 = data.tile([P, NT, C], f32)
        h2T = data.tile([P, NKC, T], bf16)
        HT = data.tile([P, NT, F], bf16)

        # ---- input DMAs: x tiles first on the two fast HWDGE queues
        # (sync/scalar) so LN1 starts ASAP; weights go on the gpsimd SWDGE
        # queue, which also leaves the gpsimd engine free afterwards ----
        x_r = x_d[:, :].rearrange("(j p) c -> p j c", p=P)
        x_engines = [nc.sync, nc.scalar, nc.sync, nc.scalar,
                     nc.sync, nc.scalar, nc.sync, nc.scalar]
        for j in range(NT):
            x_engines[j].dma_start(out=xs[:, j], in_=x_r[:, j])
        nc.gpsimd.dma_start(out=ident_s, in_=ident_d[:, :])
        nc.gpsimd.dma_start(out=wq_s, in_=wq_d[:, :].rearrange("(k p) c -> p k c", p=P))
        nc.gpsimd.dma_start(out=wk_s, in_=wk_d[:, :].rearrange("(k p) c -> p k c", p=P))
        nc.gpsimd.dma_start(out=wv_s, in_=wv_d[:, :].rearrange("(k p) c -> p k c", p=P))
        nc.gpsimd.dma_start(out=tri_s, in_=tri_d[:, :])
        nc.gpsimd.dma_start(out=wp_s, in_=wp_d[:, :].rearrange("(k p) c -> p k c", p=P))
        nc.scalar.dma_start(out=w1_s, in_=w1_d[:, :].rearrange("(k p) c -> p k c", p=P))
        nc.sync.dma_start(out=w2_s, in_=w2_d[:, :].rearrange("(k p) c -> p k c", p=P))
        if not bp_zero:
            nc.scalar.dma_start(
                out=xbp, in_=x_d[:, :].rearrange("(j p) c -> p j c", p=P))
            # add bp (broadcast along partitions) into the residual copy
            bp_b = bass.AP(tensor=bp_d, offset=0, ap=[[0, P], [1, C]])
            bpt = consts.tile([P, C], f32)
            nc.sync.dma_start(out=bpt, in_=bp_b)
            for j in range(NT):
                nc.vector.tensor_add(out=xbp[:, j], in0=xbp[:, j], in1=bpt)
        if qk_bias:
            nc.sync.dma_start(out=bq_s, in_=bq_d[:].rearrange("(m p) -> p m", p=P))
            nc.sync.dma_start(out=bk_s, in_=bk_d[:].rearrange("(m p) -> p m", p=P))
        if not b1_zero:
            nc.sync.dma_start(out=b1_s, in_=b1_d[:].rearrange("(m p) -> p m", p=P))
        if not b2_zero:
            b2t = consts.tile([P, C], f32)
            b2_b = bass.AP(tensor=b2_d, offset=0, ap=[[0, P], [1, C]])
            nc.sync.dma_start(out=b2t, in_=b2_b)

        def ln_phase(src, dst_hT, tag, tiles, sc_stats=False):
            """LayerNorm the given tiles of src [128, 8, 256] f32 and write
            the transposed bf16 result into dst_hT [128, 2, 1024].
            sc_stats: compute Σx/Σx² on ScalarE (accum_out) instead of DVE
            bn_stats — used where the DVE is the exposed critical path."""
            nj = len(tiles)
            mvall = work.tile([P, nj, 2], f32, tag="mvall", name=f"mv_{tag}")
            vpe = work.tile([P, nj], f32, tag="vpe", name=f"vpe_{tag}")
            if sc_stats:
                sx = work.tile([P, nj], f32, tag="sx", name=f"sx_{tag}")
                sxx = work.tile([P, nj], f32, tag="sxx", name=f"sxx_{tag}")
                for jx, j in enumerate(tiles):
                    scr = work.tile([P, C], f32, tag="scr")
                    nc.scalar.activation(
                        out=scr, in_=src[:, j], func=AF.Identity,
                        accum_out=sx[:, jx : jx + 1])
                    scr2 = work.tile([P, C], f32, tag="scr")
                    nc.scalar.activation(
                        out=scr2, in_=src[:, j], func=AF.Square,
                        accum_out=sxx[:, jx : jx + 1])
                nc.vector.tensor_scalar(
                    out=mvall[:, :, 0], in0=sx, scalar1=1.0 / C, scalar2=None,
                    op0=ALU.mult)
                mm = work.tile([P, nj], f32, tag="mm2", name=f"mm2_{tag}")
                nc.vector.tensor_tensor(
                    out=mm, in0=mvall[:, :, 0], in1=mvall[:, :, 0], op=ALU.mult)
                nc.vector.tensor_scalar(
                    out=vpe, in0=sxx, scalar1=1.0 / C, scalar2=EPS,
                    op0=ALU.mult, op1=ALU.add)
                nc.vector.tensor_tensor(
                    out=vpe, in0=vpe, in1=mm, op=ALU.subtract)
            else:
                for jx, j in enumerate(tiles):
                    stats = work.tile([P, 6], f32, tag="stats")
                    nc.vector.bn_stats(out=stats, in_=src[:, j])
                    nc.vector.bn_aggr(out=mvall[:, jx], in_=stats)
                nc.vector.tensor_scalar_add(
                    out=vpe, in0=mvall[:, :, 1], scalar1=EPS)
            # rstd for all tiles: Quake rsqrt + 2 Newton steps (pure DVE)
            sh = work.tile([P, nj], i32, tag="rsq_sh")
            nc.vector.tensor_scalar(
                out=sh, in0=vpe.bitcast(i32), scalar1=1, scalar2=None,
                op0=ALU.logical_shift_right,
            )
            y0 = work.tile([P, nj], i32, tag="rsq_y0")
            nc.vector.tensor_scalar(
                out=y0, in0=sh, scalar1=-1, scalar2=MAGIC,
                op0=ALU.mult, op1=ALU.add,
            )
            y = y0.bitcast(f32)
            rsq = work.tile([P, nj], f32, tag="rsq", name=f"rsq_{tag}")
            tmp = work.tile([P, nj], f32, tag="rsq_tmp")
            for it in range(1):
                nc.vector.tensor_tensor(out=tmp, in0=y, in1=y, op=ALU.mult)
                nc.vector.tensor_tensor(out=tmp, in0=tmp, in1=vpe, op=ALU.mult)
                nc.vector.tensor_scalar(
                    out=tmp, in0=tmp, scalar1=-0.5, scalar2=1.5,
                    op0=ALU.mult, op1=ALU.add,
                )
                nc.vector.tensor_tensor(out=rsq, in0=tmp, in1=y, op=ALU.mult)
                y = rsq
            # bias for the ScalarE apply: -mu*rstd
            nmurs = work.tile([P, nj], f32, tag="nmurs", name=f"nmurs_{tag}")
            nc.vector.tensor_tensor(
                out=nmurs, in0=mvall[:, :, 0], in1=rsq, op=ALU.mult)
            nc.vector.tensor_scalar(
                out=nmurs, in0=nmurs, scalar1=-1.0, scalar2=None, op0=ALU.mult)
            for jx, j in enumerate(tiles):
                hs = work.tile([P, C], bf16, tag="hstraight")
                nc.scalar.activation(
                    out=hs, in_=src[:, j], func=AF.Identity,
                    scale=rsq[:, jx : jx + 1], bias=nmurs[:, jx : jx + 1],
                )
                tp = psum.tile([P, 2, P], bf16, tag="mix", bufs=2)
                nc.tensor.transpose(tp[:, 0], hs[:, 0:P], ident_s)
                nc.tensor.transpose(tp[:, 1], hs[:, P : 2 * P], ident_s)
                nc.scalar.activation(
                    out=dst_hT[:, :, j * P : (j + 1) * P], in_=tp, func=AF.Copy)

        def qk_pack(w_s, b_s, dstT, m, c):
            ps = psum.tile([P, 512], f32, tag="mix", bufs=2)
            for k in range(NKC):
                nc.tensor.matmul(
                    ps,
                    lhsT=w_s[:, k, m * P : (m + 1) * P],
                    rhs=h1T[:, k, c * 512 : (c + 1) * 512],
                    start=(k == 0), stop=(k == NKC - 1),
                )
            if qk_bias:
                nc.vector.tensor_scalar_add(
                    out=dstT[:, m, c * 512 : (c + 1) * 512], in0=ps,
                    scalar1=b_s[:, m : m + 1],
                )
            else:
                nc.scalar.activation(
                    out=dstT[:, m, c * 512 : (c + 1) * 512], in_=ps,
                    func=AF.Copy,
                )

        def ffn1_tile(f, c):
            ps = psum.tile([P, 512], f32, tag="mix", bufs=2)
            for k in range(NKC):
                nc.tensor.matmul(
                    ps,
                    lhsT=w1_s[:, k, f * P : (f + 1) * P],
                    rhs=h2T[:, k, c * 512 : (c + 1) * 512],
                    start=(k == 0), stop=(k == NKC - 1),
                )
            if b1_zero:
                nc.scalar.activation(
                    out=HT[:, f, c * 512 : (c + 1) * 512], in_=ps, func=AF.Relu)
            else:
                nc.scalar.activation(
                    out=HT[:, f, c * 512 : (c + 1) * 512], in_=ps,
                    func=AF.Relu, bias=b1_s[:, f : f + 1],
                )

        def qk_chunk(c):
            """Q^T / K^T for tq-chunk c (padded layout, bias folded in evac).
            During chunk 0 the DVE is idle, so alternate the PSUM evacs
            between ScalarE and DVE to halve the evac critical path."""
            for ni, (name, w_s, b_s, dstT) in enumerate((
                ("q", wq_s, bq_s, QT), ("k", wk_s, bk_s, KT))):
                for m in range(NPACK):
                    ps = psum.tile([P, 512], f32, tag="mix", bufs=2)
                    for k in range(NKC):
                        nc.tensor.matmul(
                            ps,
                            lhsT=w_s[:, k, m * P : (m + 1) * P],
                            rhs=h1T[:, k, c * 512 : (c + 1) * 512],
                            start=(k == 0), stop=(k == NKC - 1),
                        )
                    dst = dstT[:, m, c * 512 : (c + 1) * 512]
                    if qk_bias:
                        nc.vector.tensor_scalar_add(
                            out=dst, in0=ps, scalar1=b_s[:, m : m + 1])
                    elif c == 0 and (ni + m) % 2 == 0:
                        nc.vector.tensor_copy(out=dst, in_=ps)
                    else:
                        nc.scalar.activation(out=dst, in_=ps, func=AF.Copy)

        def v_tiles(tiles, dve_evac=False):
            """V (straight, padded 32-wide blocks; col 16 of each = ones)."""
            for j in tiles:
                ps = psum.tile([P, 512], f32, tag="mix", bufs=2)
                for k in range(NKC):
                    nc.tensor.matmul(
                        ps,
                        lhsT=h1T[:, k, j * P : (j + 1) * P],
                        rhs=wv_s[:, k, :],
                        start=(k == 0), stop=(k == NKC - 1),
                    )
                if dve_evac and j % 2 == 0:
                    nc.vector.tensor_copy(out=Vv[:, j, :], in_=ps)
                else:
                    nc.scalar.copy(Vv[:, j, :], ps)
            ones_cols = Vv.rearrange("p j (h e) -> p j h e", e=HP)[
                :, tiles[0] : tiles[-1] + 1, :, 16:17]
            nc.vector.memset(ones_cols, 1.0)

        # fillers: independent PE work dropped into attention stall points
        fillers = []

        def emit_filler():
            if fillers:
                fillers.pop(0)()

        # ---- attention: unit = (tq-chunk, pack) ----
        def attn_unit(p, cj, fine_norm=False):
            expc = attn.tile([P, NPACK, NT, 512], bf16, tag="expc", bufs=2,
                             name=f"expc{p}_{cj}")
            tiles = list(range(0, min(NT, 4 * cj + 4)))
            # S^T as 32x32 subarray tiles; 2 heads share one 2-bank psum tile.
            # exp evac: heads 0-2 table-exp on ScalarE, head 3 Schraudolph
            # bf16-bits exp on VectorE (DVE also owns mask+normalize).
            for i in tiles:
                off = max(0, P * i - 512 * cj)  # valid start within chunk
                n = 512 - off
                # one psum alloc per head-pair: ring depth 2 pipelines
                # S^T(i+1) against the exp evac of pair (i, h01)
                for q in range(2):
                    sp = psum.tile([P, 2, 512], f32, tag="sps", bufs=2,
                                   name=f"sp{p}_{cj}_{i}_{q}")
                    for e in range(2):
                        hh = 2 * q + e
                        nc.tensor.matmul(
                            sp[:, e, 0:n],
                            lhsT=KT[HP * hh : HP * (hh + 1), p,
                                    i * P : (i + 1) * P],
                            rhs=QT[HP * hh : HP * (hh + 1), p,
                                   512 * cj + off : 512 * cj + off + n],
                            start=True, stop=True,
                            tile_position=(HP * hh, 0),
                        )
                    if q == 0:
                        nc.scalar.activation(
                            out=expc[:, 0:2, i, off : off + n],
                            in_=sp[:, :, 0:n],
                            func=AF.Exp, scale=SCALE,
                        )
                    else:
                        nc.vector.tensor_scalar(
                            out=expc[:, 2:4, i, off : off + n].bitcast(i16),
                            in0=sp[:, :, 0:n],
                            scalar1=EXP_A, scalar2=EXP_B,
                            op0=ALU.mult, op1=ALU.add,
                        )
                if i % 4 == 3:
                    emit_filler()
            # causal mask: the 8 diagonal blocks of a head-PAIR per bf16-2x
            # DVE op (two ops) so PV of pair 0 can start under pair 1's mask
            tri_r = bass.AP(
                tensor=tri_s.tensor, offset=tri_s.offset,
                ap=[list(tri_s.ap[0]), [0, 2], [0, 4], [1, P]],
            )
            for pr in range(2):
                dview = bass.AP(
                    tensor=expc.tensor,
                    offset=expc.offset + 2048 * cj + 2 * pr * NT * 512,
                    ap=[list(expc.ap[0]), [NT * 512, 2], [512 + P, 4], [1, P]],
                )
                nc.vector.tensor_tensor(
                    out=dview, in0=dview, in1=tri_r, op=ALU.mult,
                )
            # PV accumulation over valid tk tiles, head-major so each pair
            # only waits on its own mask
            pv = psum.tile([P, 512], f32, tag="pv", bufs=2, name=f"pv{p}_{cj}")
            last = max(tiles)
            for hh in range(NPACK):
                h = 4 * p + hh
                for i in tiles:
                    off = max(0, P * i - 512 * cj)
                    n = 512 - off
                    nc.tensor.matmul(
                        pv[HP * hh : HP * (hh + 1), off : off + n],
                        lhsT=Vv[:, i, HP * h : HP * (h + 1)],
                        rhs=expc[:, hh, i, off : off + n],
                        start=(i == 0), stop=(i == last),
                        tile_position=(0, HP * hh),
                        skip_group_check=True,
                    )
            # normalize: out^T = pv / Z  (Z in partition 16 of each 32-block)
            zbc = work.tile([P, 512], f32, tag="zbc")
            rz = work.tile([P, 512], f32, tag="rz")
            if fine_norm:
                # last unit: pipeline the normalize in 128-col pieces so the
                # projection (which consumes per-128-col tiles) starts early
                for s in range(4):
                    sl = slice(128 * s, 128 * (s + 1))
                    nc.vector.stream_shuffle(zbc[:, sl], pv[:, sl],
                                             mask=[16] * 32)
                    nc.vector.reciprocal_approx_fast(out=rz[:, sl],
                                                     in_=zbc[:, sl])
                    nc.vector.tensor_tensor(
                        out=OUTT[:, p, 512 * cj + 128 * s :
                                 512 * cj + 128 * (s + 1)],
                        in0=pv[:, sl], in1=rz[:, sl], op=ALU.mult,
                    )
            else:
                nc.vector.stream_shuffle(zbc, pv, mask=[16] * 32)
                nc.vector.reciprocal_approx_fast(out=rz, in_=zbc)
                nc.vector.tensor_tensor(
                    out=OUTT[:, p, 512 * cj : 512 * (cj + 1)], in0=pv, in1=rz,
                    op=ALU.mult,
                )

        def proj_tile(j):
            ps = psum.tile([P, C], f32, tag="mix", bufs=2)
            for k in range(NPACK):
                nc.tensor.matmul(
                    ps,
                    lhsT=OUTT[:, k, j * P : (j + 1) * P],
                    rhs=wp_s[:, k, :],
                    start=(k == 0), stop=(k == NPACK - 1),
                )
            nc.vector.tensor_add(out=x1[:, j], in0=ps, in1=xbp[:, j])

        def ffn1_chunk(c):
            for f in range(NT):
                ffn1_tile(f, c)

        def ffn2_tile(j):
            ps = psum.tile([P, C], f32, tag="mix", bufs=2)
            for f in range(NT):
                nc.tensor.matmul(
                    ps,
                    lhsT=HT[:, f, j * P : (j + 1) * P],
                    rhs=w2_s[:, f, :],
                    start=(f == 0), stop=(f == NT - 1),
                )
            if b2_zero:
                outs = work.tile([P, C], bf16, tag="outs")
                nc.vector.tensor_add(out=outs, in0=ps, in1=x1[:, j])
            else:
                outs32 = work.tile([P, C], f32, tag="outs32")
                nc.vector.tensor_add(out=outs32, in0=ps, in1=x1[:, j])
                outs = work.tile([P, C], bf16, tag="outs")
                nc.vector.tensor_add(out=outs, in0=outs32, in1=b2t)
            eng = nc.sync if j % 2 == 0 else nc.scalar
            eng.dma_start(
                out=out_d[:, :].rearrange("(t p) c -> p t c", p=P)[:, j], in_=outs
            )

        # ---- schedule ----
        ln_phase(xs, h1T, "ln1a0", [0])
        ln_phase(xs, h1T, "ln1a1", [1])
        ln_phase(xs, h1T, "ln1a2", [2, 3])
        qk_chunk(0)
        v_tiles([0, 1, 2, 3], dve_evac=True)
        ln_phase(xs, h1T, "ln1b", list(range(4, NT)))
        for m in range(NPACK):
            fillers.append(lambda m=m: qk_pack(wq_s, bq_s, QT, m, 1))
        for p in range(NPACK):
            attn_unit(p, 0)
        while fillers:
            fillers.pop(0)()
        for m in range(NPACK):
            qk_pack(wk_s, bk_s, KT, m, 1)
        v_tiles([4, 5, 6, 7])
        for j in range(4):
            proj_tile(j)
        ln_phase(x1, h2T, "ln2_0", list(range(4)))
        for f in range(NT):
            fillers.append(lambda f=f: ffn1_tile(f, 0))
        for p in range(NPACK):
            attn_unit(p, 1, fine_norm=(p == NPACK - 1))
        while fillers:
            fillers.pop(0)()
        for j in range(4):
            ffn2_tile(j)
        for j in range(4, NT):
            proj_tile(j)
        ln_phase(x1, h2T, "ln2_1", list(range(4, NT)))
        ffn1_chunk(1)
        for j in range(4, NT):
            ffn2_tile(j)

        for pool in (psum, work, attn, data, consts):
            pool.release()

    nc.compile()
    return nc


def _prep_inputs(x, Wq, Wk, Wv, Wp, bp, W1, b1, W2, b2, g1, be1, g2, be2):
    """Host-side preprocessing: fold LN affines into the following matmuls,
    pad per-head weights to 32-wide blocks, cast to bf16."""
    f32 = np.float32
    x = np.asarray(x, f32).astype(_BF16)
    Wqf = np.asarray(Wq, f32).reshape(C, C) * np.asarray(g1, f32)[:, None]
    Wkf = np.asarray(Wk, f32).reshape(C, C) * np.asarray(g1, f32)[:, None]
    Wvf = np.asarray(Wv, f32).reshape(C, C) * np.asarray(g1, f32)[:, None]
    bqf = np.asarray(be1, f32) @ np.asarray(Wq, f32).reshape(C, C)
    bkf = np.asarray(be1, f32) @ np.asarray(Wk, f32).reshape(C, C)
    bvf = np.asarray(be1, f32) @ np.asarray(Wv, f32).reshape(C, C)

    def pad_cols(w):
        wp = np.zeros((C, CP), f32)
        for h in range(H):
            wp[:, HP * h : HP * h + D] = w[:, D * h : D * (h + 1)]
        return wp

    def pad_vec(v):
        vp = np.zeros((CP,), f32)
        for h in range(H):
            vp[HP * h : HP * h + D] = v[D * h : D * (h + 1)]
        return vp

    wq_p = pad_cols(Wqf)
    wk_p = pad_cols(Wkf)
    wv_p = pad_cols(Wvf)
    bq_p = pad_vec(bqf)
    bk_p = pad_vec(bkf)
    bv_p = pad_vec(bvf)

    wp_p = np.zeros((CP, C), f32)
    for h in range(H):
        wp_p[HP * h : HP * h + D, :] = np.asarray(Wp, f32)[D * h : D * (h + 1), :]

    W1f = np.asarray(W1, f32) * np.asarray(g2, f32)[:, None]
    b1f = np.asarray(b1, f32) + np.asarray(be2, f32) @ np.asarray(W1, f32)

    shared = {
        "wq": wq_p.astype(_BF16), "wk": wk_p.astype(_BF16),
        "wv": wv_p.astype(_BF16), "wp": wp_p.astype(_BF16),
        "w1": W1f.astype(_BF16), "w2": np.asarray(W2, f32).astype(_BF16),
        "bq": bq_p, "bk": bk_p,
        "bprow": np.asarray(bp, f32), "b1p": b1f,
        "b2row": np.asarray(b2, f32),
    }
    assert not np.any(bv_p), "nonzero V bias not folded on-device (be1 != 0)"
    return x, shared


def kernel(**inputs) -> np.ndarray:
    from concourse import bass_utils

    x, shared = _prep_inputs(**inputs)
    qk_bias = bool(np.any(shared["bq"]) or np.any(shared["bk"]))
    bp_zero = not np.any(shared["bprow"])
    b2_zero = not np.any(shared["b2row"])
    b1_zero = not np.any(shared["b1p"])
    key = ("nc", qk_bias, bp_zero, b2_zero, b1_zero)
    if key not in _cache:
        _cache[key] = _build_program(
            qk_bias=qk_bias, bp_zero=bp_zero, b2_zero=b2_zero, b1_zero=b1_zero)
    nc = _cache[key]

    in_maps = [dict(shared, x=np.ascontiguousarray(x[i])) for i in range(B)]
    res = bass_utils.run_bass_kernel_spmd(nc, in_maps, core_ids=list(range(B)))
    _cache["last_result"] = res
    out = np.stack([r["out"] for r in res.results], axis=0)
    return out.astype(np.float32)



# revision 31
# speedup vs baseline: 1.1818x; 1.0580x over previous
"""Trainium2 Bass kernel for one pre-LN transformer block (B=8, T=1024, C=256,
H=16 heads of size 16, FFN 256->1024->256), data-parallel over batch across 8
NeuronCores (one batch element per core).

Per-core dataflow (matmul operands bf16, accumulation fp32):
  x arrives bf16 per-tile on three DMA queues (sync/scalar/gpsimd) so LN1
    starts ~9us in; LN affines are folded into the QKV/FFN1 weights host-side
  LN1 (straight [T,C]; batched Quake-rsqrt on DVE, apply+evacs on ScalarE
    with per-partition scale/bias) -> PE-transpose -> h1^T [C,T]
  Q^T/K^T in padded head layout [h*32+d, T] (pad rows zero, padded weights)
  V straight [T, h*32+{d,16=ones-col,zeros}] - the ones column makes the PV
    matmul also produce the softmax denominator (scores are tiny: no max pass)
  S^T[tk,tq] = k^T.T @ q^T per head via 32-row-strip matmuls (4 heads share
    the 128-row PE array); one PSUM alloc per head-PAIR so the ring pipelines
    S^T(i+1) against exp(i); exp fused into the PSUM->SBUF evac: heads 0-1
    table-exp on ScalarE, heads 2-3 Schraudolph bf16-bits exp on VectorE;
    causal diag blocks masked by one batched diagonal-strided triangular
    multiply per (pack, head) against a single [128,128] mask read 4x via a
    stride-0 middle AP dim
  PV: out^T[d,tq] accumulated over tk tiles with 32-col-strip matmuls
  normalize via per-head Z row broadcast (stream_shuffle from PSUM) +
    reciprocal_approx_fast + multiply (the last unit runs this in 128-col
    pieces so the projection can start early)
  proj: x1 = x(+bp) + out^T.T @ Wp   (out^T tiles are the stationary operand)
  LN2 -> h2^T -> FFN1 (relu+bias on ScalarE evac) -> FFN2 -> + x1

Scheduling: program order interleaves chunk production with attention so the
PE never waits on a full-phase barrier, and independent GEMMs are dropped
into attention stall points as "fillers" (QK chunk-1 packs into attention
chunk 0, FFN1 chunk-0 tiles into attention chunk 1) to keep the tensor
engine dense while exp evacuations drain.
"""

import os
import sys

for _p in ("/opt/trn_rl_repo", "/root/.axon_site/_ro/trn_rl_repo"):
    if os.path.isdir(_p) and _p not in sys.path:
        sys.path.append(_p)

import numpy as np
import ml_dtypes

# problem shapes (hardcoded per contest rules)
B, T, C, H, D, F = 8, 1024, 256, 16, 16, 1024
P = 128          # partitions
NT = T // P      # 8 T-tiles
HP = 32          # padded per-head stride (Q/K/V/out layouts)
CP = H * HP      # 512 padded channel dim
NPACK = 4        # head packs (4 heads per 128-partition tile)
NKC = C // P     # 2 k-tiles over C
EPS = 1e-5
SCALE = D ** -0.5
MAGIC = 0x5F3759DF
# Schraudolph-style exp to bf16 bits: bf16_bits(exp(SCALE*s)) ~= EXP_A*s + EXP_B
EXP_A = (2 ** 7) * SCALE * 1.4426950408889634
EXP_B = 2 ** 7 * 127 - 5.6

_BF16 = ml_dtypes.bfloat16

_cache = {}


def _build_program(qk_bias=False, bp_zero=False, b2_zero=False):
    import concourse.bass as bass
    import concourse.bacc as bacc
    import concourse.tile as tile
    import concourse.mybir as mybir

    dt = mybir.dt
    f32, bf16, i32, i16 = dt.float32, dt.bfloat16, dt.int32, dt.int16
    AF = mybir.ActivationFunctionType
    ALU = mybir.AluOpType

    nc = bacc.Bacc("TRN2", target_bir_lowering=False, debug=False)

    # ---- DRAM I/O ----
    x_d = nc.dram_tensor("x", [T, C], bf16, kind="ExternalInput")
    wq_d = nc.dram_tensor("wq", [C, CP], bf16, kind="ExternalInput")
    wk_d = nc.dram_tensor("wk", [C, CP], bf16, kind="ExternalInput")
    wv_d = nc.dram_tensor("wv", [C, CP], bf16, kind="ExternalInput")
    wp_d = nc.dram_tensor("wp", [CP, C], bf16, kind="ExternalInput")
    w1_d = nc.dram_tensor("w1", [C, F], bf16, kind="ExternalInput")
    w2_d = nc.dram_tensor("w2", [F, C], bf16, kind="ExternalInput")
    bq_d = nc.dram_tensor("bq", [CP], f32, kind="ExternalInput")
    bk_d = nc.dram_tensor("bk", [CP], f32, kind="ExternalInput")
    bp_d = nc.dram_tensor("bprow", [C], f32, kind="ExternalInput")
    b1_d = nc.dram_tensor("b1p", [F], f32, kind="ExternalInput")
    b2_d = nc.dram_tensor("b2row", [C], f32, kind="ExternalInput")
    out_d = nc.dram_tensor("out", [T, C], f32, kind="ExternalOutput")

    ident_np = np.eye(P, dtype=_BF16)
    # S^T diag tile mask: partition = tk local, free = tq local; keep tq >= tk
    tri_np = np.triu(np.ones((P, P), dtype=np.float32)).astype(_BF16)
    ident_d = nc.inline_tensor(ident_np, name="ident")
    tri_d = nc.inline_tensor(tri_np, name="trimask")

    with tile.TileContext(nc) as tc:
        consts = tc.alloc_tile_pool(name="consts", bufs=1)
        data = tc.alloc_tile_pool(name="data", bufs=1)
        attn = tc.alloc_tile_pool(name="attn", bufs=1)
        work = tc.alloc_tile_pool(name="work", bufs=4)
        psum = tc.alloc_tile_pool(name="psum", bufs=1, space="PSUM")

        # ---- persistent SBUF tensors ----
        ident_s = consts.tile([P, P], bf16)
        tri_s = consts.tile([P, P], bf16)
        wq_s = consts.tile([P, NKC, CP], bf16)
        wk_s = consts.tile([P, NKC, CP], bf16)
        wv_s = consts.tile([P, NKC, CP], bf16)
        wp_s = consts.tile([P, NPACK, C], bf16)
        w1_s = consts.tile([P, NKC, F], bf16)
        w2_s = consts.tile([P, NT, C], bf16)
        bq_s = consts.tile([P, NPACK], f32)
        bk_s = consts.tile([P, NPACK], f32)
        b1_s = consts.tile([P, NT], f32)

        xs = data.tile([P, NT, C], bf16)
        xbp = xs if bp_zero else data.tile([P, NT, C], bf16)
        h1T = data.tile([P, NKC, T], bf16)
        QT = data.tile([P, NPACK, T], bf16)
        KT = data.tile([P, NPACK, T], bf16)
        Vv = data.tile([P, NT, CP], bf16)
        OUTT = data.tile([P, NPACK, T], bf16)
        x1 = data.tile([P, NT, C], f32)
        h2T = data.tile([P, NKC, T], bf16)
        HT = data.tile([P, NT, F], bf16)

        # ---- input DMAs (x per-tile, spread across the three DMA-capable
        # engine queues so the first LN1 tiles land fast) ----
        x_r = x_d[:, :].rearrange("(j p) c -> p j c", p=P)
        x_engines = [nc.sync, nc.scalar, nc.sync, nc.scalar,
                     nc.sync, nc.scalar, nc.gpsimd, nc.gpsimd]
        for j in range(NT):
            x_engines[j].dma_start(out=xs[:, j], in_=x_r[:, j])
        nc.gpsimd.dma_start(out=ident_s, in_=ident_d[:, :])
        nc.gpsimd.dma_start(out=wq_s, in_=wq_d[:, :].rearrange("(k p) c -> p k c", p=P))
        nc.gpsimd.dma_start(out=wk_s, in_=wk_d[:, :].rearrange("(k p) c -> p k c", p=P))
        nc.gpsimd.dma_start(out=wv_s, in_=wv_d[:, :].rearrange("(k p) c -> p k c", p=P))
        nc.gpsimd.dma_start(out=tri_s, in_=tri_d[:, :])
        nc.scalar.dma_start(out=wp_s, in_=wp_d[:, :].rearrange("(k p) c -> p k c", p=P))
        nc.scalar.dma_start(out=w1_s, in_=w1_d[:, :].rearrange("(k p) c -> p k c", p=P))
        nc.scalar.dma_start(out=w2_s, in_=w2_d[:, :].rearrange("(k p) c -> p k c", p=P))
        if not bp_zero:
            nc.vector.dma_start(
                out=xbp, in_=x_d[:, :].rearrange("(j p) c -> p j c", p=P))
            # add bp (broadcast along partitions) into the residual copy
            bp_b = bass.AP(tensor=bp_d, offset=0, ap=[[0, P], [1, C]])
            bpt = consts.tile([P, C], f32)
            nc.sync.dma_start(out=bpt, in_=bp_b)
            for j in range(NT):
                nc.vector.tensor_add(out=xbp[:, j], in0=xbp[:, j], in1=bpt)
        nc.sync.dma_start(out=bq_s, in_=bq_d[:].rearrange("(m p) -> p m", p=P))
        nc.sync.dma_start(out=bk_s, in_=bk_d[:].rearrange("(m p) -> p m", p=P))
        nc.sync.dma_start(out=b1_s, in_=b1_d[:].rearrange("(m p) -> p m", p=P))
        b2t = consts.tile([P, C], f32)
        b2_b = bass.AP(tensor=b2_d, offset=0, ap=[[0, P], [1, C]])
        nc.sync.dma_start(out=b2t, in_=b2_b)

        def ln_phase(src, dst_hT, tag, tiles, sc_stats=False):
            """LayerNorm the given tiles of src [128, 8, 256] f32 and write
            the transposed bf16 result into dst_hT [128, 2, 1024].
            sc_stats: compute Σx/Σx² on ScalarE (accum_out) instead of DVE
            bn_stats — used where the DVE is the exposed critical path."""
            nj = len(tiles)
            mvall = work.tile([P, nj, 2], f32, tag="mvall", name=f"mv_{tag}")
            vpe = work.tile([P, nj], f32, tag="vpe", name=f"vpe_{tag}")
            if sc_stats:
                sx = work.tile([P, nj], f32, tag="sx", name=f"sx_{tag}")
                sxx = work.tile([P, nj], f32, tag="sxx", name=f"sxx_{tag}")
                for jx, j in enumerate(tiles):
                    scr = work.tile([P, C], f32, tag="scr")
                    nc.scalar.activation(
                        out=scr, in_=src[:, j], func=AF.Identity,
                        accum_out=sx[:, jx : jx + 1])
                    scr2 = work.tile([P, C], f32, tag="scr")
                    nc.scalar.activation(
                        out=scr2, in_=src[:, j], func=AF.Square,
                        accum_out=sxx[:, jx : jx + 1])
                nc.vector.tensor_scalar(
                    out=mvall[:, :, 0], in0=sx, scalar1=1.0 / C, scalar2=None,
                    op0=ALU.mult)
                mm = work.tile([P, nj], f32, tag="mm2", name=f"mm2_{tag}")
                nc.vector.tensor_tensor(
                    out=mm, in0=mvall[:, :, 0], in1=mvall[:, :, 0], op=ALU.mult)
                nc.vector.tensor_scalar(
                    out=vpe, in0=sxx, scalar1=1.0 / C, scalar2=EPS,
                    op0=ALU.mult, op1=ALU.add)
                nc.vector.tensor_tensor(
                    out=vpe, in0=vpe, in1=mm, op=ALU.subtract)
            else:
                for jx, j in enumerate(tiles):
                    stats = work.tile([P, 6], f32, tag="stats")
                    nc.vector.bn_stats(out=stats, in_=src[:, j])
                    nc.vector.bn_aggr(out=mvall[:, jx], in_=stats)
                nc.vector.tensor_scalar_add(
                    out=vpe, in0=mvall[:, :, 1], scalar1=EPS)
            # rstd for all tiles: Quake rsqrt + 2 Newton steps (pure DVE)
            sh = work.tile([P, nj], i32, tag="rsq_sh")
            nc.vector.tensor_scalar(
                out=sh, in0=vpe.bitcast(i32), scalar1=1, scalar2=None,
                op0=ALU.logical_shift_right,
            )
            y0 = work.tile([P, nj], i32, tag="rsq_y0")
            nc.vector.tensor_scalar(
                out=y0, in0=sh, scalar1=-1, scalar2=MAGIC,
                op0=ALU.mult, op1=ALU.add,
            )
            y = y0.bitcast(f32)
            rsq = work.tile([P, nj], f32, tag="rsq", name=f"rsq_{tag}")
            tmp = work.tile([P, nj], f32, tag="rsq_tmp")
            for it in range(2):
                nc.vector.tensor_tensor(out=tmp, in0=y, in1=y, op=ALU.mult)
                nc.vector.tensor_tensor(out=tmp, in0=tmp, in1=vpe, op=ALU.mult)
                nc.vector.tensor_scalar(
                    out=tmp, in0=tmp, scalar1=-0.5, scalar2=1.5,
                    op0=ALU.mult, op1=ALU.add,
                )
                nc.vector.tensor_tensor(out=rsq, in0=tmp, in1=y, op=ALU.mult)
                y = rsq
            # bias for the ScalarE apply: -mu*rstd
            nmurs = work.tile([P, nj], f32, tag="nmurs", name=f"nmurs_{tag}")
            nc.vector.tensor_tensor(
                out=nmurs, in0=mvall[:, :, 0], in1=rsq, op=ALU.mult)
            nc.vector.tensor_scalar(
                out=nmurs, in0=nmurs, scalar1=-1.0, scalar2=None, op0=ALU.mult)
            for jx, j in enumerate(tiles):
                hs = work.tile([P, C], bf16, tag="hstraight")
                nc.scalar.activation(
                    out=hs, in_=src[:, j], func=AF.Identity,
                    scale=rsq[:, jx : jx + 1], bias=nmurs[:, jx : jx + 1],
                )
                tp = psum.tile([P, 2, P], bf16, tag="mix", bufs=2)
                nc.tensor.transpose(tp[:, 0], hs[:, 0:P], ident_s)
                nc.tensor.transpose(tp[:, 1], hs[:, P : 2 * P], ident_s)
                nc.scalar.activation(
                    out=dst_hT[:, :, j * P : (j + 1) * P], in_=tp, func=AF.Copy)

        def qk_pack(w_s, b_s, dstT, m, c):
            ps = psum.tile([P, 512], f32, tag="mix", bufs=2)
            for k in range(NKC):
                nc.tensor.matmul(
                    ps,
                    lhsT=w_s[:, k, m * P : (m + 1) * P],
                    rhs=h1T[:, k, c * 512 : (c + 1) * 512],
                    start=(k == 0), stop=(k == NKC - 1),
                )
            if qk_bias:
                nc.vector.tensor_scalar_add(
                    out=dstT[:, m, c * 512 : (c + 1) * 512], in0=ps,
                    scalar1=b_s[:, m : m + 1],
                )
            else:
                nc.scalar.activation(
                    out=dstT[:, m, c * 512 : (c + 1) * 512], in_=ps,
                    func=AF.Copy,
                )

        def ffn1_tile(f, c):
            ps = psum.tile([P, 512], f32, tag="mix", bufs=2)
            for k in range(NKC):
                nc.tensor.matmul(
                    ps,
                    lhsT=w1_s[:, k, f * P : (f + 1) * P],
                    rhs=h2T[:, k, c * 512 : (c + 1) * 512],
                    start=(k == 0), stop=(k == NKC - 1),
                )
            nc.scalar.activation(
                out=HT[:, f, c * 512 : (c + 1) * 512], in_=ps,
                func=AF.Relu, bias=b1_s[:, f : f + 1],
            )

        def qk_chunk(c):
            """Q^T / K^T for tq-chunk c (padded layout, bias folded in evac)."""
            for (name, w_s, b_s, dstT) in (
                ("q", wq_s, bq_s, QT), ("k", wk_s, bk_s, KT)):
                for m in range(NPACK):
                    ps = psum.tile([P, 512], f32, tag="mix", bufs=2)
                    for k in range(NKC):
                        nc.tensor.matmul(
                            ps,
                            lhsT=w_s[:, k, m * P : (m + 1) * P],
                            rhs=h1T[:, k, c * 512 : (c + 1) * 512],
                            start=(k == 0), stop=(k == NKC - 1),
                        )
                    if qk_bias:
                        nc.vector.tensor_scalar_add(
                            out=dstT[:, m, c * 512 : (c + 1) * 512], in0=ps,
                            scalar1=b_s[:, m : m + 1],
                        )
                    else:
                        nc.scalar.activation(
                            out=dstT[:, m, c * 512 : (c + 1) * 512], in_=ps,
                            func=AF.Copy,
                        )

        def v_tiles(tiles):
            """V (straight, padded 32-wide blocks; col 16 of each = ones)."""
            for j in tiles:
                ps = psum.tile([P, 512], f32, tag="mix", bufs=2)
                for k in range(NKC):
                    nc.tensor.matmul(
                        ps,
                        lhsT=h1T[:, k, j * P : (j + 1) * P],
                        rhs=wv_s[:, k, :],
                        start=(k == 0), stop=(k == NKC - 1),
                    )
                nc.scalar.copy(Vv[:, j, :], ps)
            ones_cols = Vv.rearrange("p j (h e) -> p j h e", e=HP)[
                :, tiles[0] : tiles[-1] + 1, :, 16:17]
            nc.vector.memset(ones_cols, 1.0)

        # fillers: independent PE work dropped into attention stall points
        fillers = []

        def emit_filler():
            if fillers:
                fillers.pop(0)()

        # ---- attention: unit = (tq-chunk, pack) ----
        def attn_unit(p, cj, fine_norm=False):
            expc = attn.tile([P, NPACK, NT, 512], bf16, tag="expc", bufs=2,
                             name=f"expc{p}_{cj}")
            tiles = list(range(0, min(NT, 4 * cj + 4)))
            # S^T as 32x32 subarray tiles; 2 heads share one 2-bank psum tile.
            # exp evac: heads 0-2 table-exp on ScalarE, head 3 Schraudolph
            # bf16-bits exp on VectorE (DVE also owns mask+normalize).
            for i in tiles:
                off = max(0, P * i - 512 * cj)  # valid start within chunk
                n = 512 - off
                # one psum alloc per head-pair: ring depth 2 pipelines
                # S^T(i+1) against the exp evac of pair (i, h01)
                for q in range(2):
                    sp = psum.tile([P, 2, 512], f32, tag="sps", bufs=2,
                                   name=f"sp{p}_{cj}_{i}_{q}")
                    for e in range(2):
                        hh = 2 * q + e
                        nc.tensor.matmul(
                            sp[:, e, 0:n],
                            lhsT=KT[HP * hh : HP * (hh + 1), p,
                                    i * P : (i + 1) * P],
                            rhs=QT[HP * hh : HP * (hh + 1), p,
                                   512 * cj + off : 512 * cj + off + n],
                            start=True, stop=True,
                            tile_position=(HP * hh, 0),
                        )
                    if q == 0:
                        nc.scalar.activation(
                            out=expc[:, 0:2, i, off : off + n],
                            in_=sp[:, :, 0:n],
                            func=AF.Exp, scale=SCALE,
                        )
                    else:
                        nc.vector.tensor_scalar(
                            out=expc[:, 2:4, i, off : off + n].bitcast(i16),
                            in0=sp[:, :, 0:n],
                            scalar1=EXP_A, scalar2=EXP_B,
                            op0=ALU.mult, op1=ALU.add,
                        )
                if i % 4 == 3:
                    emit_filler()
            # causal mask: the 4 diagonal blocks of this chunk per head
            tri_r = bass.AP(
                tensor=tri_s.tensor, offset=tri_s.offset,
                ap=[list(tri_s.ap[0]), [0, 4], [1, P]],
            )
            for hh in range(NPACK):
                base = expc[:, hh]
                dview = bass.AP(
                    tensor=base.tensor,
                    offset=base.offset + 2048 * cj,
                    ap=[list(base.ap[0]), [512 + P, 4], [1, P]],
                )
                nc.vector.tensor_tensor(
                    out=dview, in0=dview, in1=tri_r, op=ALU.mult,
                )
            # PV accumulation over valid tk tiles
            pv = psum.tile([P, 512], f32, tag="pv", bufs=2, name=f"pv{p}_{cj}")
            last = max(tiles)
            for i in tiles:
                off = max(0, P * i - 512 * cj)
                n = 512 - off
                for hh in range(NPACK):
                    h = 4 * p + hh
                    nc.tensor.matmul(
                        pv[HP * hh : HP * (hh + 1), off : off + n],
                        lhsT=Vv[:, i, HP * h : HP * (h + 1)],
                        rhs=expc[:, hh, i, off : off + n],
                        start=(i == 0), stop=(i == last),
                        tile_position=(0, HP * hh),
                        skip_group_check=True,
                    )
            # normalize: out^T = pv / Z  (Z in partition 16 of each 32-block)
            zbc = work.tile([P, 512], f32, tag="zbc")
            rz = work.tile([P, 512], f32, tag="rz")
            if fine_norm:
                # last unit: pipeline the normalize in 128-col pieces so the
                # projection (which consumes per-128-col tiles) starts early
                for s in range(4):
                    sl = slice(128 * s, 128 * (s + 1))
                    nc.vector.stream_shuffle(zbc[:, sl], pv[:, sl],
                                             mask=[16] * 32)
                    nc.vector.reciprocal_approx_fast(out=rz[:, sl],
                                                     in_=zbc[:, sl])
                    nc.vector.tensor_tensor(
                        out=OUTT[:, p, 512 * cj + 128 * s :
                                 512 * cj + 128 * (s + 1)],
                        in0=pv[:, sl], in1=rz[:, sl], op=ALU.mult,
                    )
            else:
                nc.vector.stream_shuffle(zbc, pv, mask=[16] * 32)
                nc.vector.reciprocal_approx_fast(out=rz, in_=zbc)
                nc.vector.tensor_tensor(
                    out=OUTT[:, p, 512 * cj : 512 * (cj + 1)], in0=pv, in1=rz,
                    op=ALU.mult,
                )

        def proj_tile(j):
            ps = psum.tile([P, C], f32, tag="mix", bufs=2)
            for k in range(NPACK):
                nc.tensor.matmul(
                    ps,
                    lhsT=OUTT[:, k, j * P : (j + 1) * P],
                    rhs=wp_s[:, k, :],
                    start=(k == 0), stop=(k == NPACK - 1),
                )
            nc.vector.tensor_add(out=x1[:, j], in0=ps, in1=xbp[:, j])

        def ffn1_chunk(c):
            for f in range(NT):
                ps = psum.tile([P, 512], f32, tag="mix", bufs=2)
                for k in range(NKC):
                    nc.tensor.matmul(
                        ps,
                        lhsT=w1_s[:, k, f * P : (f + 1) * P],
                        rhs=h2T[:, k, c * 512 : (c + 1) * 512],
                        start=(k == 0), stop=(k == NKC - 1),
                    )
                nc.scalar.activation(
                    out=HT[:, f, c * 512 : (c + 1) * 512], in_=ps,
                    func=AF.Relu, bias=b1_s[:, f : f + 1],
                )

        def ffn2_tile(j):
            ps = psum.tile([P, C], f32, tag="mix", bufs=2)
            for f in range(NT):
                nc.tensor.matmul(
                    ps,
                    lhsT=HT[:, f, j * P : (j + 1) * P],
                    rhs=w2_s[:, f, :],
                    start=(f == 0), stop=(f == NT - 1),
                )
            outs = work.tile([P, C], f32, tag="outs")
            nc.vector.tensor_add(out=outs, in0=ps, in1=x1[:, j])
            if not b2_zero:
                nc.vector.tensor_add(out=outs, in0=outs, in1=b2t)
            nc.sync.dma_start(
                out=out_d[:, :].rearrange("(t p) c -> p t c", p=P)[:, j], in_=outs
            )

        # ---- schedule ----
        ln_phase(xs, h1T, "ln1a0", [0, 1])
        ln_phase(xs, h1T, "ln1a1", [2, 3])
        qk_chunk(0)
        v_tiles([0, 1, 2, 3])
        ln_phase(xs, h1T, "ln1b", list(range(4, NT)))
        for m in range(NPACK):
            fillers.append(lambda m=m: qk_pack(wq_s, bq_s, QT, m, 1))
        for p in range(NPACK):
            attn_unit(p, 0)
        while fillers:
            fillers.pop(0)()
        for m in range(NPACK):
            qk_pack(wk_s, bk_s, KT, m, 1)
        v_tiles([4, 5, 6, 7])
        for j in range(4):
            proj_tile(j)
        ln_phase(x1, h2T, "ln2_0", list(range(4)))
        for f in range(NT):
            fillers.append(lambda f=f: ffn1_tile(f, 0))
        for p in range(NPACK):
            attn_unit(p, 1, fine_norm=(p == NPACK - 1))
        while fillers:
            fillers.pop(0)()
        for j in range(4):
            ffn2_tile(j)
        for j in range(4, NT):
            proj_tile(j)
        ln_phase(x1, h2T, "ln2_1", list(range(4, NT)))
        ffn1_chunk(1)
        for j in range(4, NT):
            ffn2_tile(j)

        for pool in (psum, work, attn, data, consts):
            pool.release()

    nc.compile()
    return nc


def _prep_inputs(x, Wq, Wk, Wv, Wp, bp, W1, b1, W2, b2, g1, be1, g2, be2):
    """Host-side preprocessing: fold LN affines into the following matmuls,
    pad per-head weights to 32-wide blocks, cast to bf16."""
    f32 = np.float32
    x = np.asarray(x, f32).astype(_BF16)
    Wqf = np.asarray(Wq, f32).reshape(C, C) * np.asarray(g1, f32)[:, None]
    Wkf = np.asarray(Wk, f32).reshape(C, C) * np.asarray(g1, f32)[:, None]
    Wvf = np.asarray(Wv, f32).reshape(C, C) * np.asarray(g1, f32)[:, None]
    bqf = np.asarray(be1, f32) @ np.asarray(Wq, f32).reshape(C, C)
    bkf = np.asarray(be1, f32) @ np.asarray(Wk, f32).reshape(C, C)
    bvf = np.asarray(be1, f32) @ np.asarray(Wv, f32).reshape(C, C)

    def pad_cols(w):
        wp = np.zeros((C, CP), f32)
        for h in range(H):
            wp[:, HP * h : HP * h + D] = w[:, D * h : D * (h + 1)]
        return wp

    def pad_vec(v):
        vp = np.zeros((CP,), f32)
        for h in range(H):
            vp[HP * h : HP * h + D] = v[D * h : D * (h + 1)]
        return vp

    wq_p = pad_cols(Wqf)
    wk_p = pad_cols(Wkf)
    wv_p = pad_cols(Wvf)
    bq_p = pad_vec(bqf)
    bk_p = pad_vec(bkf)
    bv_p = pad_vec(bvf)

    wp_p = np.zeros((CP, C), f32)
    for h in range(H):
        wp_p[HP * h : HP * h + D, :] = np.asarray(Wp, f32)[D * h : D * (h + 1), :]

    W1f = np.asarray(W1, f32) * np.asarray(g2, f32)[:, None]
    b1f = np.asarray(b1, f32) + np.asarray(be2, f32) @ np.asarray(W1, f32)

    shared = {
        "wq": wq_p.astype(_BF16), "wk": wk_p.astype(_BF16),
        "wv": wv_p.astype(_BF16), "wp": wp_p.astype(_BF16),
        "w1": W1f.astype(_BF16), "w2": np.asarray(W2, f32).astype(_BF16),
        "bq": bq_p, "bk": bk_p,
        "bprow": np.asarray(bp, f32), "b1p": b1f,
        "b2row": np.asarray(b2, f32),
    }
    assert not np.any(bv_p), "nonzero V bias not folded on-device (be1 != 0)"
    return x, shared


def kernel(**inputs) -> np.ndarray:
    from concourse import bass_utils

    x, shared = _prep_inputs(**inputs)
    qk_bias = bool(np.any(shared["bq"]) or np.any(shared["bk"]))
    bp_zero = not np.any(shared["bprow"])
    b2_zero = not np.any(shared["b2row"])
    key = ("nc", qk_bias, bp_zero, b2_zero)
    if key not in _cache:
        _cache[key] = _build_program(
            qk_bias=qk_bias, bp_zero=bp_zero, b2_zero=b2_zero)
    nc = _cache[key]

    in_maps = [dict(shared, x=np.ascontiguousarray(x[i])) for i in range(B)]
    res = bass_utils.run_bass_kernel_spmd(nc, in_maps, core_ids=list(range(B)))
    _cache["last_result"] = res
    out = np.stack([r["out"] for r in res.results], axis=0)
    return out.astype(np.float32)



# revision 33
# speedup vs baseline: 1.2381x; 1.0476x over previous
"""Trainium2 Bass kernel for one pre-LN transformer block (B=8, T=1024, C=256,
H=16 heads of size 16, FFN 256->1024->256), data-parallel over batch across 8
NeuronCores (one batch element per core).

Per-core dataflow (matmul operands bf16, accumulation fp32):
  x arrives bf16 per-tile on three DMA queues (sync/scalar/gpsimd) so LN1
    starts ~9us in; LN affines are folded into the QKV/FFN1 weights host-side
  LN1 (straight [T,C]; batched Quake-rsqrt on DVE, apply+evacs on ScalarE
    with per-partition scale/bias) -> PE-transpose -> h1^T [C,T]
  Q^T/K^T in padded head layout [h*32+d, T] (pad rows zero, padded weights)
  V straight [T, h*32+{d,16=ones-col,zeros}] - the ones column makes the PV
    matmul also produce the softmax denominator (scores are tiny: no max pass)
  S^T[tk,tq] = k^T.T @ q^T per head via 32-row-strip matmuls (4 heads share
    the 128-row PE array); one PSUM alloc per head-PAIR so the ring pipelines
    S^T(i+1) against exp(i); exp fused into the PSUM->SBUF evac: heads 0-1
    table-exp on ScalarE, heads 2-3 Schraudolph bf16-bits exp on VectorE;
    causal diag blocks masked by one batched diagonal-strided triangular
    multiply per (pack, head) against a single [128,128] mask read 4x via a
    stride-0 middle AP dim
  PV: out^T[d,tq] accumulated over tk tiles with 32-col-strip matmuls
  normalize via per-head Z row broadcast (stream_shuffle from PSUM) +
    reciprocal_approx_fast + multiply (the last unit runs this in 128-col
    pieces so the projection can start early)
  proj: x1 = x(+bp) + out^T.T @ Wp   (out^T tiles are the stationary operand)
  LN2 -> h2^T -> FFN1 (relu+bias on ScalarE evac) -> FFN2 -> + x1

Scheduling: program order interleaves chunk production with attention so the
PE never waits on a full-phase barrier, and independent GEMMs are dropped
into attention stall points as "fillers" (QK chunk-1 packs into attention
chunk 0, FFN1 chunk-0 tiles into attention chunk 1) to keep the tensor
engine dense while exp evacuations drain.
"""

import os
import sys

for _p in ("/opt/trn_rl_repo", "/root/.axon_site/_ro/trn_rl_repo"):
    if os.path.isdir(_p) and _p not in sys.path:
        sys.path.append(_p)

import numpy as np
import ml_dtypes

# problem shapes (hardcoded per contest rules)
B, T, C, H, D, F = 8, 1024, 256, 16, 16, 1024
P = 128          # partitions
NT = T // P      # 8 T-tiles
HP = 32          # padded per-head stride (Q/K/V/out layouts)
CP = H * HP      # 512 padded channel dim
NPACK = 4        # head packs (4 heads per 128-partition tile)
NKC = C // P     # 2 k-tiles over C
EPS = 1e-5
SCALE = D ** -0.5
MAGIC = 0x5F3759DF
# Schraudolph-style exp to bf16 bits: bf16_bits(exp(SCALE*s)) ~= EXP_A*s + EXP_B
EXP_A = (2 ** 7) * SCALE * 1.4426950408889634
EXP_B = 2 ** 7 * 127 - 5.6

_BF16 = ml_dtypes.bfloat16

_cache = {}


def _build_program(qk_bias=False, bp_zero=False, b2_zero=False):
    import concourse.bass as bass
    import concourse.bacc as bacc
    import concourse.tile as tile
    import concourse.mybir as mybir

    dt = mybir.dt
    f32, bf16, i32, i16 = dt.float32, dt.bfloat16, dt.int32, dt.int16
    AF = mybir.ActivationFunctionType
    ALU = mybir.AluOpType

    nc = bacc.Bacc("TRN2", target_bir_lowering=False, debug=False)

    # ---- DRAM I/O ----
    x_d = nc.dram_tensor("x", [T, C], bf16, kind="ExternalInput")
    wq_d = nc.dram_tensor("wq", [C, CP], bf16, kind="ExternalInput")
    wk_d = nc.dram_tensor("wk", [C, CP], bf16, kind="ExternalInput")
    wv_d = nc.dram_tensor("wv", [C, CP], bf16, kind="ExternalInput")
    wp_d = nc.dram_tensor("wp", [CP, C], bf16, kind="ExternalInput")
    w1_d = nc.dram_tensor("w1", [C, F], bf16, kind="ExternalInput")
    w2_d = nc.dram_tensor("w2", [F, C], bf16, kind="ExternalInput")
    bq_d = nc.dram_tensor("bq", [CP], f32, kind="ExternalInput")
    bk_d = nc.dram_tensor("bk", [CP], f32, kind="ExternalInput")
    bp_d = nc.dram_tensor("bprow", [C], f32, kind="ExternalInput")
    b1_d = nc.dram_tensor("b1p", [F], f32, kind="ExternalInput")
    b2_d = nc.dram_tensor("b2row", [C], f32, kind="ExternalInput")
    out_d = nc.dram_tensor("out", [T, C], bf16, kind="ExternalOutput")

    ident_np = np.eye(P, dtype=_BF16)
    # S^T diag tile mask: partition = tk local, free = tq local; keep tq >= tk
    tri_np = np.triu(np.ones((P, P), dtype=np.float32)).astype(_BF16)
    ident_d = nc.inline_tensor(ident_np, name="ident")
    tri_d = nc.inline_tensor(tri_np, name="trimask")

    with tile.TileContext(nc) as tc:
        consts = tc.alloc_tile_pool(name="consts", bufs=1)
        data = tc.alloc_tile_pool(name="data", bufs=1)
        attn = tc.alloc_tile_pool(name="attn", bufs=1)
        work = tc.alloc_tile_pool(name="work", bufs=4)
        psum = tc.alloc_tile_pool(name="psum", bufs=1, space="PSUM")

        # ---- persistent SBUF tensors ----
        ident_s = consts.tile([P, P], bf16)
        tri_s = consts.tile([P, P], bf16)
        wq_s = consts.tile([P, NKC, CP], bf16)
        wk_s = consts.tile([P, NKC, CP], bf16)
        wv_s = consts.tile([P, NKC, CP], bf16)
        wp_s = consts.tile([P, NPACK, C], bf16)
        w1_s = consts.tile([P, NKC, F], bf16)
        w2_s = consts.tile([P, NT, C], bf16)
        bq_s = consts.tile([P, NPACK], f32)
        bk_s = consts.tile([P, NPACK], f32)
        b1_s = consts.tile([P, NT], f32)

        xs = data.tile([P, NT, C], bf16)
        xbp = xs if bp_zero else data.tile([P, NT, C], bf16)
        h1T = data.tile([P, NKC, T], bf16)
        QT = data.tile([P, NPACK, T], bf16)
        KT = data.tile([P, NPACK, T], bf16)
        Vv = data.tile([P, NT, CP], bf16)
        OUTT = data.tile([P, NPACK, T], bf16)
        x1 = data.tile([P, NT, C], f32)
        h2T = data.tile([P, NKC, T], bf16)
        HT = data.tile([P, NT, F], bf16)

        # ---- input DMAs (x per-tile, spread across the three DMA-capable
        # engine queues so the first LN1 tiles land fast) ----
        x_r = x_d[:, :].rearrange("(j p) c -> p j c", p=P)
        x_engines = [nc.sync, nc.scalar, nc.sync, nc.scalar,
                     nc.sync, nc.scalar, nc.gpsimd, nc.gpsimd]
        for j in range(NT):
            x_engines[j].dma_start(out=xs[:, j], in_=x_r[:, j])
        nc.gpsimd.dma_start(out=ident_s, in_=ident_d[:, :])
        nc.gpsimd.dma_start(out=wq_s, in_=wq_d[:, :].rearrange("(k p) c -> p k c", p=P))
        nc.gpsimd.dma_start(out=wk_s, in_=wk_d[:, :].rearrange("(k p) c -> p k c", p=P))
        nc.gpsimd.dma_start(out=wv_s, in_=wv_d[:, :].rearrange("(k p) c -> p k c", p=P))
        nc.gpsimd.dma_start(out=tri_s, in_=tri_d[:, :])
        nc.scalar.dma_start(out=wp_s, in_=wp_d[:, :].rearrange("(k p) c -> p k c", p=P))
        nc.scalar.dma_start(out=w1_s, in_=w1_d[:, :].rearrange("(k p) c -> p k c", p=P))
        nc.scalar.dma_start(out=w2_s, in_=w2_d[:, :].rearrange("(k p) c -> p k c", p=P))
        if not bp_zero:
            nc.vector.dma_start(
                out=xbp, in_=x_d[:, :].rearrange("(j p) c -> p j c", p=P))
            # add bp (broadcast along partitions) into the residual copy
            bp_b = bass.AP(tensor=bp_d, offset=0, ap=[[0, P], [1, C]])
            bpt = consts.tile([P, C], f32)
            nc.sync.dma_start(out=bpt, in_=bp_b)
            for j in range(NT):
                nc.vector.tensor_add(out=xbp[:, j], in0=xbp[:, j], in1=bpt)
        if qk_bias:
            nc.sync.dma_start(out=bq_s, in_=bq_d[:].rearrange("(m p) -> p m", p=P))
            nc.sync.dma_start(out=bk_s, in_=bk_d[:].rearrange("(m p) -> p m", p=P))
        nc.sync.dma_start(out=b1_s, in_=b1_d[:].rearrange("(m p) -> p m", p=P))
        if not b2_zero:
            b2t = consts.tile([P, C], f32)
            b2_b = bass.AP(tensor=b2_d, offset=0, ap=[[0, P], [1, C]])
            nc.sync.dma_start(out=b2t, in_=b2_b)

        def ln_phase(src, dst_hT, tag, tiles, sc_stats=False):
            """LayerNorm the given tiles of src [128, 8, 256] f32 and write
            the transposed bf16 result into dst_hT [128, 2, 1024].
            sc_stats: compute Σx/Σx² on ScalarE (accum_out) instead of DVE
            bn_stats — used where the DVE is the exposed critical path."""
            nj = len(tiles)
            mvall = work.tile([P, nj, 2], f32, tag="mvall", name=f"mv_{tag}")
            vpe = work.tile([P, nj], f32, tag="vpe", name=f"vpe_{tag}")
            if sc_stats:
                sx = work.tile([P, nj], f32, tag="sx", name=f"sx_{tag}")
                sxx = work.tile([P, nj], f32, tag="sxx", name=f"sxx_{tag}")
                for jx, j in enumerate(tiles):
                    scr = work.tile([P, C], f32, tag="scr")
                    nc.scalar.activation(
                        out=scr, in_=src[:, j], func=AF.Identity,
                        accum_out=sx[:, jx : jx + 1])
                    scr2 = work.tile([P, C], f32, tag="scr")
                    nc.scalar.activation(
                        out=scr2, in_=src[:, j], func=AF.Square,
                        accum_out=sxx[:, jx : jx + 1])
                nc.vector.tensor_scalar(
                    out=mvall[:, :, 0], in0=sx, scalar1=1.0 / C, scalar2=None,
                    op0=ALU.mult)
                mm = work.tile([P, nj], f32, tag="mm2", name=f"mm2_{tag}")
                nc.vector.tensor_tensor(
                    out=mm, in0=mvall[:, :, 0], in1=mvall[:, :, 0], op=ALU.mult)
                nc.vector.tensor_scalar(
                    out=vpe, in0=sxx, scalar1=1.0 / C, scalar2=EPS,
                    op0=ALU.mult, op1=ALU.add)
                nc.vector.tensor_tensor(
                    out=vpe, in0=vpe, in1=mm, op=ALU.subtract)
            else:
                for jx, j in enumerate(tiles):
                    stats = work.tile([P, 6], f32, tag="stats")
                    nc.vector.bn_stats(out=stats, in_=src[:, j])
                    nc.vector.bn_aggr(out=mvall[:, jx], in_=stats)
                nc.vector.tensor_scalar_add(
                    out=vpe, in0=mvall[:, :, 1], scalar1=EPS)
            # rstd for all tiles: Quake rsqrt + 2 Newton steps (pure DVE)
            sh = work.tile([P, nj], i32, tag="rsq_sh")
            nc.vector.tensor_scalar(
                out=sh, in0=vpe.bitcast(i32), scalar1=1, scalar2=None,
                op0=ALU.logical_shift_right,
            )
            y0 = work.tile([P, nj], i32, tag="rsq_y0")
            nc.vector.tensor_scalar(
                out=y0, in0=sh, scalar1=-1, scalar2=MAGIC,
                op0=ALU.mult, op1=ALU.add,
            )
            y = y0.bitcast(f32)
            rsq = work.tile([P, nj], f32, tag="rsq", name=f"rsq_{tag}")
            tmp = work.tile([P, nj], f32, tag="rsq_tmp")
            for it in range(1):
                nc.vector.tensor_tensor(out=tmp, in0=y, in1=y, op=ALU.mult)
                nc.vector.tensor_tensor(out=tmp, in0=tmp, in1=vpe, op=ALU.mult)
                nc.vector.tensor_scalar(
                    out=tmp, in0=tmp, scalar1=-0.5, scalar2=1.5,
                    op0=ALU.mult, op1=ALU.add,
                )
                nc.vector.tensor_tensor(out=rsq, in0=tmp, in1=y, op=ALU.mult)
                y = rsq
            # bias for the ScalarE apply: -mu*rstd
            nmurs = work.tile([P, nj], f32, tag="nmurs", name=f"nmurs_{tag}")
            nc.vector.tensor_tensor(
                out=nmurs, in0=mvall[:, :, 0], in1=rsq, op=ALU.mult)
            nc.vector.tensor_scalar(
                out=nmurs, in0=nmurs, scalar1=-1.0, scalar2=None, op0=ALU.mult)
            for jx, j in enumerate(tiles):
                hs = work.tile([P, C], bf16, tag="hstraight")
                nc.scalar.activation(
                    out=hs, in_=src[:, j], func=AF.Identity,
                    scale=rsq[:, jx : jx + 1], bias=nmurs[:, jx : jx + 1],
                )
                tp = psum.tile([P, 2, P], bf16, tag="mix", bufs=2)
                nc.tensor.transpose(tp[:, 0], hs[:, 0:P], ident_s)
                nc.tensor.transpose(tp[:, 1], hs[:, P : 2 * P], ident_s)
                nc.scalar.activation(
                    out=dst_hT[:, :, j * P : (j + 1) * P], in_=tp, func=AF.Copy)

        def qk_pack(w_s, b_s, dstT, m, c):
            ps = psum.tile([P, 512], f32, tag="mix", bufs=2)
            for k in range(NKC):
                nc.tensor.matmul(
                    ps,
                    lhsT=w_s[:, k, m * P : (m + 1) * P],
                    rhs=h1T[:, k, c * 512 : (c + 1) * 512],
                    start=(k == 0), stop=(k == NKC - 1),
                )
            if qk_bias:
                nc.vector.tensor_scalar_add(
                    out=dstT[:, m, c * 512 : (c + 1) * 512], in0=ps,
                    scalar1=b_s[:, m : m + 1],
                )
            else:
                nc.scalar.activation(
                    out=dstT[:, m, c * 512 : (c + 1) * 512], in_=ps,
                    func=AF.Copy,
                )

        def ffn1_tile(f, c):
            ps = psum.tile([P, 512], f32, tag="mix", bufs=2)
            for k in range(NKC):
                nc.tensor.matmul(
                    ps,
                    lhsT=w1_s[:, k, f * P : (f + 1) * P],
                    rhs=h2T[:, k, c * 512 : (c + 1) * 512],
                    start=(k == 0), stop=(k == NKC - 1),
                )
            nc.scalar.activation(
                out=HT[:, f, c * 512 : (c + 1) * 512], in_=ps,
                func=AF.Relu, bias=b1_s[:, f : f + 1],
            )

        def qk_chunk(c):
            """Q^T / K^T for tq-chunk c (padded layout, bias folded in evac)."""
            for (name, w_s, b_s, dstT) in (
                ("q", wq_s, bq_s, QT), ("k", wk_s, bk_s, KT)):
                for m in range(NPACK):
                    ps = psum.tile([P, 512], f32, tag="mix", bufs=2)
                    for k in range(NKC):
                        nc.tensor.matmul(
                            ps,
                            lhsT=w_s[:, k, m * P : (m + 1) * P],
                            rhs=h1T[:, k, c * 512 : (c + 1) * 512],
                            start=(k == 0), stop=(k == NKC - 1),
                        )
                    if qk_bias:
                        nc.vector.tensor_scalar_add(
                            out=dstT[:, m, c * 512 : (c + 1) * 512], in0=ps,
                            scalar1=b_s[:, m : m + 1],
                        )
                    elif c == 0 and m % 2 == 1:
                        nc.vector.tensor_copy(
                            out=dstT[:, m, c * 512 : (c + 1) * 512], in_=ps)
                    else:
                        nc.scalar.activation(
                            out=dstT[:, m, c * 512 : (c + 1) * 512], in_=ps,
                            func=AF.Copy,
                        )

        def v_tiles(tiles):
            """V (straight, padded 32-wide blocks; col 16 of each = ones)."""
            for j in tiles:
                ps = psum.tile([P, 512], f32, tag="mix", bufs=2)
                for k in range(NKC):
                    nc.tensor.matmul(
                        ps,
                        lhsT=h1T[:, k, j * P : (j + 1) * P],
                        rhs=wv_s[:, k, :],
                        start=(k == 0), stop=(k == NKC - 1),
                    )
                if j < 4 and j % 2 == 1:
                    nc.vector.tensor_copy(out=Vv[:, j, :], in_=ps)
                else:
                    nc.scalar.copy(Vv[:, j, :], ps)
            ones_cols = Vv.rearrange("p j (h e) -> p j h e", e=HP)[
                :, tiles[0] : tiles[-1] + 1, :, 16:17]
            nc.vector.memset(ones_cols, 1.0)

        # fillers: independent PE work dropped into attention stall points
        fillers = []

        def emit_filler():
            if fillers:
                fillers.pop(0)()

        # ---- attention: unit = (tq-chunk, pack) ----
        def attn_unit(p, cj, fine_norm=False):
            expc = attn.tile([P, NPACK, NT, 512], bf16, tag="expc", bufs=2,
                             name=f"expc{p}_{cj}")
            tiles = list(range(0, min(NT, 4 * cj + 4)))
            # S^T as 32x32 subarray tiles; 2 heads share one 2-bank psum tile.
            # exp evac: heads 0-2 table-exp on ScalarE, head 3 Schraudolph
            # bf16-bits exp on VectorE (DVE also owns mask+normalize).
            for i in tiles:
                off = max(0, P * i - 512 * cj)  # valid start within chunk
                n = 512 - off
                # one psum alloc per head-pair: ring depth 2 pipelines
                # S^T(i+1) against the exp evac of pair (i, h01)
                for q in range(2):
                    sp = psum.tile([P, 2, 512], f32, tag="sps", bufs=2,
                                   name=f"sp{p}_{cj}_{i}_{q}")
                    for e in range(2):
                        hh = 2 * q + e
                        nc.tensor.matmul(
                            sp[:, e, 0:n],
                            lhsT=KT[HP * hh : HP * (hh + 1), p,
                                    i * P : (i + 1) * P],
                            rhs=QT[HP * hh : HP * (hh + 1), p,
                                   512 * cj + off : 512 * cj + off + n],
                            start=True, stop=True,
                            tile_position=(HP * hh, 0),
                        )
                    if q == 0:
                        nc.scalar.activation(
                            out=expc[:, 0:2, i, off : off + n],
                            in_=sp[:, :, 0:n],
                            func=AF.Exp, scale=SCALE,
                        )
                    else:
                        nc.vector.tensor_scalar(
                            out=expc[:, 2:4, i, off : off + n].bitcast(i16),
                            in0=sp[:, :, 0:n],
                            scalar1=EXP_A, scalar2=EXP_B,
                            op0=ALU.mult, op1=ALU.add,
                        )
                if i % 4 == 3:
                    emit_filler()
            # causal mask: the 4 diagonal blocks of this chunk per head
            tri_r = bass.AP(
                tensor=tri_s.tensor, offset=tri_s.offset,
                ap=[list(tri_s.ap[0]), [0, 4], [1, P]],
            )
            for hh in range(NPACK):
                base = expc[:, hh]
                dview = bass.AP(
                    tensor=base.tensor,
                    offset=base.offset + 2048 * cj,
                    ap=[list(base.ap[0]), [512 + P, 4], [1, P]],
                )
                nc.vector.tensor_tensor(
                    out=dview, in0=dview, in1=tri_r, op=ALU.mult,
                )
            # PV accumulation over valid tk tiles
            pv = psum.tile([P, 512], f32, tag="pv", bufs=2, name=f"pv{p}_{cj}")
            last = max(tiles)
            for i in tiles:
                off = max(0, P * i - 512 * cj)
                n = 512 - off
                for hh in range(NPACK):
                    h = 4 * p + hh
                    nc.tensor.matmul(
                        pv[HP * hh : HP * (hh + 1), off : off + n],
                        lhsT=Vv[:, i, HP * h : HP * (h + 1)],
                        rhs=expc[:, hh, i, off : off + n],
                        start=(i == 0), stop=(i == last),
                        tile_position=(0, HP * hh),
                        skip_group_check=True,
                    )
            # normalize: out^T = pv / Z  (Z in partition 16 of each 32-block)
            zbc = work.tile([P, 512], f32, tag="zbc")
            rz = work.tile([P, 512], f32, tag="rz")
            if fine_norm:
                # last unit: pipeline the normalize in 128-col pieces so the
                # projection (which consumes per-128-col tiles) starts early
                for s in range(4):
                    sl = slice(128 * s, 128 * (s + 1))
                    nc.vector.stream_shuffle(zbc[:, sl], pv[:, sl],
                                             mask=[16] * 32)
                    nc.vector.reciprocal_approx_fast(out=rz[:, sl],
                                                     in_=zbc[:, sl])
                    nc.vector.tensor_tensor(
                        out=OUTT[:, p, 512 * cj + 128 * s :
                                 512 * cj + 128 * (s + 1)],
                        in0=pv[:, sl], in1=rz[:, sl], op=ALU.mult,
                    )
            else:
                nc.vector.stream_shuffle(zbc, pv, mask=[16] * 32)
                nc.vector.reciprocal_approx_fast(out=rz, in_=zbc)
                nc.vector.tensor_tensor(
                    out=OUTT[:, p, 512 * cj : 512 * (cj + 1)], in0=pv, in1=rz,
                    op=ALU.mult,
                )

        def proj_tile(j):
            ps = psum.tile([P, C], f32, tag="mix", bufs=2)
            for k in range(NPACK):
                nc.tensor.matmul(
                    ps,
                    lhsT=OUTT[:, k, j * P : (j + 1) * P],
                    rhs=wp_s[:, k, :],
                    start=(k == 0), stop=(k == NPACK - 1),
                )
            nc.vector.tensor_add(out=x1[:, j], in0=ps, in1=xbp[:, j])

        def ffn1_chunk(c):
            for f in range(NT):
                ps = psum.tile([P, 512], f32, tag="mix", bufs=2)
                for k in range(NKC):
                    nc.tensor.matmul(
                        ps,
                        lhsT=w1_s[:, k, f * P : (f + 1) * P],
                        rhs=h2T[:, k, c * 512 : (c + 1) * 512],
                        start=(k == 0), stop=(k == NKC - 1),
                    )
                nc.scalar.activation(
                    out=HT[:, f, c * 512 : (c + 1) * 512], in_=ps,
                    func=AF.Relu, bias=b1_s[:, f : f + 1],
                )

        def ffn2_tile(j):
            ps = psum.tile([P, C], f32, tag="mix", bufs=2)
            for f in range(NT):
                nc.tensor.matmul(
                    ps,
                    lhsT=HT[:, f, j * P : (j + 1) * P],
                    rhs=w2_s[:, f, :],
                    start=(f == 0), stop=(f == NT - 1),
                )
            if b2_zero:
                outs = work.tile([P, C], bf16, tag="outs")
                nc.vector.tensor_add(out=outs, in0=ps, in1=x1[:, j])
            else:
                outs32 = work.tile([P, C], f32, tag="outs32")
                nc.vector.tensor_add(out=outs32, in0=ps, in1=x1[:, j])
                outs = work.tile([P, C], bf16, tag="outs")
                nc.vector.tensor_add(out=outs, in0=outs32, in1=b2t)
            eng = nc.sync if j % 2 == 0 else nc.scalar
            eng.dma_start(
                out=out_d[:, :].rearrange("(t p) c -> p t c", p=P)[:, j], in_=outs
            )

        # ---- schedule ----
        ln_phase(xs, h1T, "ln1a0", [0, 1])
        ln_phase(xs, h1T, "ln1a1", [2, 3])
        qk_chunk(0)
        v_tiles([0, 1, 2, 3])
        ln_phase(xs, h1T, "ln1b", list(range(4, NT)))
        for m in range(NPACK):
            fillers.append(lambda m=m: qk_pack(wq_s, bq_s, QT, m, 1))
        for p in range(NPACK):
            attn_unit(p, 0)
        while fillers:
            fillers.pop(0)()
        for m in range(NPACK):
            qk_pack(wk_s, bk_s, KT, m, 1)
        v_tiles([4, 5, 6, 7])
        for j in range(4):
            proj_tile(j)
        ln_phase(x1, h2T, "ln2_0", list(range(4)))
        for f in range(NT):
            fillers.append(lambda f=f: ffn1_tile(f, 0))
        for p in range(NPACK):
            attn_unit(p, 1, fine_norm=(p == NPACK - 1))
        while fillers:
            fillers.pop(0)()
        for j in range(4):
            ffn2_tile(j)
        for j in range(4, NT):
            proj_tile(j)
        ln_phase(x1, h2T, "ln2_1", list(range(4, NT)))
        ffn1_chunk(1)
        for j in range(4, NT):
            ffn2_tile(j)

        for pool in (psum, work, attn, data, consts):
            pool.release()

    nc.compile()
    return nc


def _prep_inputs(x, Wq, Wk, Wv, Wp, bp, W1, b1, W2, b2, g1, be1, g2, be2):
    """Host-side preprocessing: fold LN affines into the following matmuls,
    pad per-head weights to 32-wide blocks, cast to bf16."""
    f32 = np.float32
    x = np.asarray(x, f32).astype(_BF16)
    Wqf = np.asarray(Wq, f32).reshape(C, C) * np.asarray(g1, f32)[:, None]
    Wkf = np.asarray(Wk, f32).reshape(C, C) * np.asarray(g1, f32)[:, None]
    Wvf = np.asarray(Wv, f32).reshape(C, C) * np.asarray(g1, f32)[:, None]
    bqf = np.asarray(be1, f32) @ np.asarray(Wq, f32).reshape(C, C)
    bkf = np.asarray(be1, f32) @ np.asarray(Wk, f32).reshape(C, C)
    bvf = np.asarray(be1, f32) @ np.asarray(Wv, f32).reshape(C, C)

    def pad_cols(w):
        wp = np.zeros((C, CP), f32)
        for h in range(H):
            wp[:, HP * h : HP * h + D] = w[:, D * h : D * (h + 1)]
        return wp

    def pad_vec(v):
        vp = np.zeros((CP,), f32)
        for h in range(H):
            vp[HP * h : HP * h + D] = v[D * h : D * (h + 1)]
        return vp

    wq_p = pad_cols(Wqf)
    wk_p = pad_cols(Wkf)
    wv_p = pad_cols(Wvf)
    bq_p = pad_vec(bqf)
    bk_p = pad_vec(bkf)
    bv_p = pad_vec(bvf)

    wp_p = np.zeros((CP, C), f32)
    for h in range(H):
        wp_p[HP * h : HP * h + D, :] = np.asarray(Wp, f32)[D * h : D * (h + 1), :]

    W1f = np.asarray(W1, f32) * np.asarray(g2, f32)[:, None]
    b1f = np.asarray(b1, f32) + np.asarray(be2, f32) @ np.asarray(W1, f32)

    shared = {
        "wq": wq_p.astype(_BF16), "wk": wk_p.astype(_BF16),
        "wv": wv_p.astype(_BF16), "wp": wp_p.astype(_BF16),
        "w1": W1f.astype(_BF16), "w2": np.asarray(W2, f32).astype(_BF16),
        "bq": bq_p, "bk": bk_p,
        "bprow": np.asarray(bp, f32), "b1p": b1f,
        "b2row": np.asarray(b2, f32),
    }
    assert not np.any(bv_p), "nonzero V bias not folded on-device (be1 != 0)"
    return x, shared


def kernel(**inputs) -> np.ndarray:
    from concourse import bass_utils

    x, shared = _prep_inputs(**inputs)
    qk_bias = bool(np.any(shared["bq"]) or np.any(shared["bk"]))
    bp_zero = not np.any(shared["bprow"])
    b2_zero = not np.any(shared["b2row"])
    key = ("nc", qk_bias, bp_zero, b2_zero)
    if key not in _cache:
        _cache[key] = _build_program(
            qk_bias=qk_bias, bp_zero=bp_zero, b2_zero=b2_zero)
    nc = _cache[key]

    in_maps = [dict(shared, x=np.ascontiguousarray(x[i])) for i in range(B)]
    res = bass_utils.run_bass_kernel_spmd(nc, in_maps, core_ids=list(range(B)))
    _cache["last_result"] = res
    out = np.stack([r["out"] for r in res.results], axis=0)
    return out.astype(np.float32)

